# revision 10
# baseline (speedup 1.0000x reference)
"""Self-contained GCN encoder kernel for 8 TRN2 NeuronCores (Bass/Tile).

kernel(**inputs) takes the FULL unsharded inputs (as from setup_inputs())
and returns the FULL [50000, 64] float32 output.

Strategy: stage 1 (embedding + W1) is REPLICATED on every core via a fused
host-precomputed lookup table emb_ab = emb_a@W1[:64] (+) emb_b@W1[64:128]
(one dma_gather per 8-tile group, accumulated into PSUM with an
identity-matmul, plus the numeric-feature matmul), writing the full
dinv-scaled h1 table to local DRAM -- no first AllGather, so the slow
startup CC barrier overlaps compute.  Conv aggregations shard dst-node
tiles across cores (LPT-balanced, quantile-matched slot order keeps the
SPMD stream identical); per-edge rows are fetched with dma_gather striped
over 4 SWDGE queues (4x descriptor-generation throughput) using a
mid-table base pointer and signed int16 indices (no A/B table split).
Seg-reduction is one-hot (is_equal) S-matrices x gathered rows on the
TensorEngine into PSUM; the symmetric norm is folded into table rows (src)
and the epilogue scale (dst); conv1 self-loop rows ride along as an extra
gather chunk, conv2 self-loop terms are stashed in SBUF from the conv1
epilogue.  One AllGather (h2 table) runs between the convs.
"""
import numpy as np
from concourse import bacc, mybir, tile
from concourse.bass_utils import run_bass_kernel_spmd
from concourse.masks import make_identity

P = 128
CORES = 8
N = 50000
NTILES = 392
NPAD = NTILES * P      # 50176
TPC = NTILES // CORES  # 49
NLOC = TPC * P         # 6272
C1 = 128
C2 = 64
EMB_MID = 25000
T1_MID = NPAD // 2     # 25088
T2_MID = NPAD // 2
PAD_DSTL = 30000.0
GS = 8                 # tiles per stage-1 gather op
NQ = 4                 # SWDGE queues

f32 = mybir.dt.float32
bf16 = mybir.dt.bfloat16
i16 = mybir.dt.int16


def wrap_idx(arr):
    return arr.reshape(-1, 16).T


def rup(x, m):
    return int((x + m - 1) // m * m)


def prep(x, edge_index, emb_a, emb_b, W1, b1, W2, b2):
    import ml_dtypes
    x = np.asarray(x)
    src, dst = np.asarray(edge_index[0]).astype(np.int64), \
        np.asarray(edge_index[1]).astype(np.int64)
    deg = np.bincount(dst, minlength=N).astype(np.float32) + 1.0
    dinv = np.ones(NPAD, dtype=np.float32)
    dinv[:N] = 1.0 / np.sqrt(deg)

    # ---- tile -> core assignment (LPT on edge counts) ----
    t_of_e = dst // P
    tile_cnt = np.bincount(t_of_e, minlength=NTILES)
    order = np.argsort(-tile_cnt, kind="stable")
    core_loads = np.zeros(CORES, dtype=np.int64)
    core_tiles = [[] for _ in range(CORES)]
    for t in order:
        c = int(np.argmin(core_loads))
        core_tiles[c].append(int(t))
        core_loads[c] += tile_cnt[t]
    c_of_t = np.zeros(NTILES, dtype=np.int64)
    k_of_t = np.zeros(NTILES, dtype=np.int64)
    for c in range(CORES):
        for k, t in enumerate(core_tiles[c]):
            c_of_t[t] = c
            k_of_t[t] = k

    node_ids = np.arange(NPAD)
    trow2 = c_of_t[node_ids // P] * NLOC + k_of_t[node_ids // P] * P \
        + node_ids % P

    # ---- sort edges by (core, slot) ----
    key = c_of_t[t_of_e] * TPC + k_of_t[t_of_e]
    sort = np.argsort(key, kind="stable")
    src_s = src[sort]
    trow2_s = trow2[src_s]
    dstl_s = (dst % P).astype(np.float32)[sort]
    bounds = np.searchsorted(key[sort], np.arange(CORES * TPC + 1))

    # ---- op schedule: one op per slot k; nch = max over cores ----
    nch_of_k = []
    for k in range(TPC):
        m = max(int(bounds[c * TPC + k + 1] - bounds[c * TPC + k])
                for c in range(CORES))
        nch_of_k.append(max(1, rup(m, P) // P))
    NPAIRS = sum(nch_of_k)
    NCH1 = max(nch_of_k) + 1      # +1 self chunk
    NCH2 = max(nch_of_k)
    G1COLS = sum((1 + nch) * P for nch in nch_of_k) // 16
    G2COLS = sum(nch * P for nch in nch_of_k) // 16

    # ---- per-core gather idx / dstl arrays ----
    in_maps = []
    iota = np.tile(np.arange(P, dtype=np.float32)[None, :], (P, 1))

    codes_a = np.zeros(NPAD, dtype=np.int64)
    codes_a[:N] = x[:, 0].astype(np.int64)
    codes_b = np.zeros(NPAD, dtype=np.int64)
    codes_b[:N] = x[:, 1].astype(np.int64)
    # stage-1 idx list: 49 ops x GS tiles; idx = cat_a (>=0, no trailing issue)
    eidx = np.tile(wrap_idx(codes_a.astype(np.int16)), (8, 1))

    # small gather table: emb_a@W1lo [1000, 128]; emb_b part via one-hot matmul
    emb_aw = (np.asarray(emb_a, np.float32)
              @ np.asarray(W1, np.float32)[0:64]).astype(ml_dtypes.bfloat16)
    W1Bp = (np.asarray(emb_b, np.float32)
            @ np.asarray(W1, np.float32)[64:128]).astype(ml_dtypes.bfloat16)
    xbT_rep = np.tile(codes_b.astype(np.float32)[None, :],
                      (50, 1)).astype(ml_dtypes.bfloat16)
    iotap = np.arange(P, dtype=np.float32)[:, None].astype(ml_dtypes.bfloat16)

    xT = np.zeros((8, NPAD), dtype=np.float32)
    xT[:, :N] = x[:, 2:10].T
    xT = xT.astype(ml_dtypes.bfloat16)

    dinv_all = dinv.reshape(NTILES, P).T.copy()   # [P, NTILES]

    for c in range(CORES):
        g1 = np.zeros(G1COLS * 16, dtype=np.int64)
        g2 = np.zeros(G2COLS * 16, dtype=np.int64)
        dstlm = np.full((P, NPAIRS), PAD_DSTL, dtype=np.float32)
        o1 = o2 = 0
        pc = 0
        for k in range(TPC):
            nch = nch_of_k[k]
            t = core_tiles[c][k]
            # conv1 self chunk: own tile rows
            g1[o1:o1 + P] = t * P + np.arange(P) - T1_MID
            lo, hi = bounds[c * TPC + k], bounds[c * TPC + k + 1]
            m = int(hi - lo)
            i1 = np.zeros(nch * P, dtype=np.int64)
            i2 = np.zeros(nch * P, dtype=np.int64)
            dl = np.full(nch * P, PAD_DSTL, dtype=np.float32)
            i1[:m] = src_s[lo:hi] - T1_MID
            i2[:m] = trow2_s[lo:hi] - T2_MID
            dl[:m] = dstl_s[lo:hi]
            # ensure last wrapped element (list[-1]) is >= 0 in both lists
            if i1[-1] < 0 or i2[-1] < 0:
                ok = np.where((i1 >= 0) & (i2 >= 0))[0]
                assert len(ok), "no safe trailing idx in op"
                p_ = int(ok[0])
                for arr in (i1, i2, dl):
                    arr[p_], arr[-1] = arr[-1], arr[p_]
            g1[o1 + P:o1 + P + nch * P] = i1
            g2[o2:o2 + nch * P] = i2
            for j in range(nch):
                dstlm[:, pc + j] = dl[j * P:(j + 1) * P]
            o1 += (1 + nch) * P
            o2 += nch * P
            pc += nch
        assert o1 == G1COLS * 16 and o2 == G2COLS * 16 and pc == NPAIRS

        # self-chunk trailing check: self idx can be negative only if the
        # slot's op list ends with it -- never (edge chunks follow; nch>=1)
        gidx1 = np.tile(wrap_idx(g1.astype(np.int16)), (8, 1))
        gidx2 = np.tile(wrap_idx(g2.astype(np.int16)), (8, 1))

        nodes_own = np.concatenate(
            [t * P + np.arange(P) for t in core_tiles[c]])
        dinvk = dinv[nodes_own].reshape(TPC, P).T.copy()

        rdk = (1.0 / dinvk).reshape(1, -1, order="F").astype(np.float32)

        in_maps.append({
            "emb_aw": emb_aw, "xbT_rep": xbT_rep,
            "iotap": iotap,
            "wcomb": np.concatenate([
                W1Bp.astype(np.float32),
                np.zeros((14, C1), np.float32),
                np.asarray(W1, np.float32)[128:136]]).astype(ml_dtypes.bfloat16),
            "xT": xT,
            "eidx": eidx.copy(),
            "gidx1": gidx1,
            "gidx2": gidx2,
            "dstlm": dstlm.astype(ml_dtypes.bfloat16), "dstlf": dstlm,
            "dinv_all": dinv_all,
            "dinvk": dinvk,
            "W2": np.asarray(W2, dtype=np.float32),
            "b1f": np.tile(np.asarray(b1, np.float32)[None, :], (P, 1)),
            "b2f": np.tile(np.asarray(b2, np.float32)[None, :], (P, 1)),
            "iota": iota, "iotab": iota.astype(ml_dtypes.bfloat16), "rdk": rdk,
        })

    meta = {"nch_of_k": tuple(nch_of_k), "NPAIRS": NPAIRS, "NCH1": NCH1,
            "NCH2": NCH2, "G1COLS": G1COLS, "G2COLS": G2COLS,
            "core_tiles": core_tiles}
    return in_maps, meta


def build(meta):
    nch_of_k = meta["nch_of_k"]
    NPAIRS = meta["NPAIRS"]
    NCH1 = meta["NCH1"]
    NCH2 = meta["NCH2"]
    G1COLS = meta["G1COLS"]
    G2COLS = meta["G2COLS"]
    ECOLS = NTILES * P // 16

    nc = bacc.Bacc("TRN2", target_bir_lowering=False, debug=False,
                   num_devices=CORES, num_swdge_queues=NQ)
    emb_aw = nc.dram_tensor("emb_aw", [1000, C1], bf16, kind="ExternalInput")
    wcomb = nc.dram_tensor("wcomb", [72, C1], bf16, kind="ExternalInput")
    xbT_rep = nc.dram_tensor("xbT_rep", [50, NPAD], bf16, kind="ExternalInput")
    iotap = nc.dram_tensor("iotap", [P, 1], bf16, kind="ExternalInput")
    xT = nc.dram_tensor("xT", [8, NPAD], bf16, kind="ExternalInput")
    eidx = nc.dram_tensor("eidx", [P, ECOLS], i16, kind="ExternalInput")
    gidx1 = nc.dram_tensor("gidx1", [P, G1COLS], i16, kind="ExternalInput")
    gidx2 = nc.dram_tensor("gidx2", [P, G2COLS], i16, kind="ExternalInput")
    dstlm = nc.dram_tensor("dstlm", [P, NPAIRS], bf16, kind="ExternalInput")
    iotab = nc.dram_tensor("iotab", [P, P], bf16, kind="ExternalInput")
    dstlf = nc.dram_tensor("dstlf", [P, NPAIRS], f32, kind="ExternalInput")
    rdk = nc.dram_tensor("rdk", [1, NLOC], f32, kind="ExternalInput")
    dinv_all = nc.dram_tensor("dinv_all", [P, NTILES], f32, kind="ExternalInput")
    dinvk = nc.dram_tensor("dinvk", [P, TPC], f32, kind="ExternalInput")
    W2 = nc.dram_tensor("W2", [C1, C2], f32, kind="ExternalInput")
    b1f = nc.dram_tensor("b1f", [P, C1], f32, kind="ExternalInput")
    b2f = nc.dram_tensor("b2f", [P, C2], f32, kind="ExternalInput")
    iota = nc.dram_tensor("iota", [P, P], f32, kind="ExternalInput")
    y = nc.dram_tensor("y", [NLOC, C2], f32, kind="ExternalOutput")

    with tile.TileContext(nc) as tc:
        with tc.tile_pool(name="const", bufs=1) as cpool, \
             tc.tile_pool(name="meta", bufs=1) as mpool, \
             tc.tile_pool(name="ge", bufs=4) as gepool, \
             tc.tile_pool(name="xt", bufs=3) as xtpool, \
             tc.tile_pool(name="he1", bufs=8) as he1pool, \
             tc.tile_pool(name="he2", bufs=8) as he2pool, \
             tc.tile_pool(name="sel", bufs=6) as spool, \
             tc.tile_pool(name="epi", bufs=3) as tpool, \
             tc.tile_pool(name="stash", bufs=1) as stpool, \
             tc.tile_pool(name="ptr", bufs=1, space="PSUM") as ptrp, \
             tc.tile_pool(name="pmm", bufs=2, space="PSUM") as pmmp, \
             tc.tile_pool(name="pacc", bufs=5, space="PSUM") as paccp, \
             tc.tile_pool(name="dram", bufs=1, space="DRAM") as dram:

            # ---------- constants ----------
            ident = cpool.tile([P, P], f32, tag="ident")
            make_identity(nc, ident[:])
            identb = cpool.tile([P, P], bf16, tag="identb")
            nc.vector.tensor_copy(out=identb[:], in_=ident[:])
            iota_t = cpool.tile([P, P], f32, tag="iota")
            nc.sync.dma_start(out=iota_t[:], in_=iota[:])
            iotab_t = cpool.tile([P, P], bf16, tag="iotab")
            nc.sync.dma_start(out=iotab_t[:], in_=iotab[:])
            iotap_t = cpool.tile([P, 1], bf16, tag="iotap")
            nc.sync.dma_start(out=iotap_t[:], in_=iotap[:])
            rdk_t = cpool.tile([1, NLOC], f32, tag="rdk")
            nc.sync.dma_start(out=rdk_t[:], in_=rdk[:])
            wcomb_t = cpool.tile([72, C1], bf16, tag="wcomb")
            nc.sync.dma_start(out=wcomb_t[:], in_=wcomb[:])
            W2t = cpool.tile([C1, C2], f32, tag="w2")
            nc.sync.dma_start(out=W2t[:], in_=W2[:])
            b1t = cpool.tile([P, C1], f32, tag="b1")
            nc.sync.dma_start(out=b1t[:], in_=b1f[:])
            b2t = cpool.tile([P, C2], f32, tag="b2")
            nc.sync.dma_start(out=b2t[:], in_=b2f[:])
            dinvA = cpool.tile([P, NTILES], f32, tag="dinvA")
            nc.sync.dma_start(out=dinvA[:], in_=dinv_all[:])
            dinvK = cpool.tile([P, TPC], f32, tag="dinvK")
            nc.sync.dma_start(out=dinvK[:], in_=dinvk[:])
            eidx_t = mpool.tile([P, ECOLS], i16, tag="eidx")
            nc.sync.dma_start(out=eidx_t[:], in_=eidx[:])
            h2stash = stpool.tile([P, TPC * C2], f32, tag="h2stash")

            table1 = dram.tile([NPAD, C1], bf16, tag="table1")
            ag2 = dram.tile([NLOC, C2], f32, tag="ag2")
            table2 = dram.tile([NPAD, C2], f32, tag="table2")

            gq = [0]

            def next_q():
                q = gq[0] % NQ
                gq[0] += 1
                return q

            # ---------- stage 1 (replicated): build full h1 table ----------
            for e in range(NTILES // GS):
                nidx = GS * P
                ge = gepool.tile([P, GS * P], bf16, tag="ge", name=f"ge_{e}")
                nc.gpsimd.dma_gather(
                    out_ap=ge[:].rearrange("p (n c) -> p n c", c=C1),
                    in_ap=emb_aw[:],
                    idxs_ap=eidx_t[:, e * nidx // 16:(e + 1) * nidx // 16],
                    num_idxs=nidx, num_idxs_reg=nidx, elem_size=C1,
                    single_packet=False, queue_num=next_q())
                xb_c = xtpool.tile([50, GS * P], bf16, tag="xb", name=f"xb_{e}")
                nc.sync.dma_start(out=xb_c[:],
                                  in_=xbT_rep[:, e * GS * P:(e + 1) * GS * P])
                comb = gepool.tile([72, GS * P], bf16, tag="ob", name=f"ob_{e}")
                nc.sync.dma_start(out=comb[64:72, :],
                                  in_=xT[:, e * GS * P:(e + 1) * GS * P])
                nc.vector.memset(comb[32:64, :], 0.0)
                nc.vector.tensor_tensor(
                    out=comb[0:50, :], in0=xb_c[:],
                    in1=iotap_t[0:50, 0:1].to_broadcast([50, GS * P]),
                    op=mybir.AluOpType.is_equal)
                h1st = xtpool.tile([P, GS * C1], bf16, tag="h1st",
                                   name=f"h1st_{e}")
                for half in range(GS // 4):
                    w = e * (GS // 4) + half
                    php = pmmp if w % 2 == 0 else paccp
                    wide = php.tile([P, 4 * C1], f32, space="PSUM",
                                    tag="pmm" if w % 2 == 0 else "pacc",
                                    name=f"wide_{w}")
                    nc.tensor.matmul(out=wide[:], lhsT=identb[:],
                                     rhs=ge[:, half * 4 * C1:(half + 1) * 4 * C1],
                                     start=True, stop=False)
                    for jj in range(4):
                        j = half * 4 + jj
                        t = e * GS + j
                        nc.tensor.matmul(
                            out=wide[:, jj * C1:(jj + 1) * C1],
                            lhsT=comb[:, j * P:(j + 1) * P],
                            rhs=wcomb_t[:], start=False, stop=True)
                    for jj in range(4):
                        j = half * 4 + jj
                        t = e * GS + j
                        if t % 2 == 0:
                            nc.scalar.activation(
                                out=h1st[:, j * C1:(j + 1) * C1],
                                in_=wide[:, jj * C1:(jj + 1) * C1],
                                func=mybir.ActivationFunctionType.Copy,
                                scale=dinvA[:, t:t + 1])
                        else:
                            nc.vector.tensor_tensor(
                                out=h1st[:, j * C1:(j + 1) * C1],
                                in0=wide[:, jj * C1:(jj + 1) * C1],
                                in1=dinvA[:, t:t + 1].to_broadcast([P, C1]),
                                op=mybir.AluOpType.mult)
                nc.sync.dma_start(
                    out=table1[e * GS * P:(e + 1) * GS * P, :].rearrange(
                        "(n p) c -> p n c", p=P),
                    in_=h1st[:].rearrange("p (n c) -> p n c", c=C1))

            # conv metadata loads (overlap stage-1)
            gidx1_t = mpool.tile([P, G1COLS], i16, tag="gidx1")
            nc.sync.dma_start(out=gidx1_t[:], in_=gidx1[:])
            gidx2_t = mpool.tile([P, G2COLS], i16, tag="gidx2")
            nc.sync.dma_start(out=gidx2_t[:], in_=gidx2[:])
            dstl_t = mpool.tile([P, NPAIRS], bf16, tag="dstl")
            nc.sync.dma_start(out=dstl_t[:], in_=dstlm[:])
            dstlf_t = mpool.tile([P, NPAIRS], f32, tag="dstlf")
            nc.sync.dma_start(out=dstlf_t[:], in_=dstlf[:])

            tc.strict_bb_all_engine_barrier()

            # ---------- conv1 ----------
            o1 = 0
            pc = 0
            for k in range(TPC):
                nch = nch_of_k[k]
                nidx = (1 + nch) * P
                he = he1pool.tile([P, NCH1 * C1], bf16, tag="he1",
                                  name=f"he1_{k}")
                nc.gpsimd.dma_gather(
                    out_ap=he[:, 0:(1 + nch) * C1].rearrange(
                        "p (n c) -> p n c", c=C1),
                    in_ap=table1[T1_MID:, :],
                    idxs_ap=gidx1_t[:, o1 // 16:(o1 + nidx) // 16],
                    num_idxs=nidx, num_idxs_reg=nidx, elem_size=C1,
                    single_packet=False, queue_num=next_q())
                o1 += nidx
                pacc = paccp.tile([P, C1], f32, space="PSUM", tag="pacc",
                                  name=f"pacc1_{k}")
                for j in range(nch):
                    S = spool.tile([P, P], bf16, tag="S1", name=f"S1_{k}_{j}")
                    nc.vector.tensor_tensor(
                        out=S[:],
                        in0=dstl_t[:, pc + j:pc + j + 1].to_broadcast([P, P]),
                        in1=iotab_t[:], op=mybir.AluOpType.is_equal)
                    nc.tensor.matmul(
                        out=pacc[:], lhsT=S[:],
                        rhs=he[:, (1 + j) * C1:(2 + j) * C1],
                        start=(j == 0), stop=False)
                pc += nch
                # pacc += self rows; pacc += b1/dinv (so relu(dinv*pacc) is exact)
                nc.tensor.matmul(out=pacc[:], lhsT=identb[:], rhs=he[:, 0:C1],
                                 start=False, stop=False)
                nc.tensor.matmul(out=pacc[:], lhsT=rdk_t[:, k * P:(k + 1) * P],
                                 rhs=b1t[0:1, :], start=False, stop=True)
                t4 = tpool.tile([P, C1], f32, tag="t4", name=f"t4_{k}")
                nc.scalar.activation(out=t4[:], in_=pacc[:],
                                     func=mybir.ActivationFunctionType.Relu,
                                     scale=dinvK[:, k:k + 1])
                # h2 = (t4 @ W2) * dinv
                ptr2 = ptrp.tile([P, P], f32, space="PSUM", tag="ptr",
                                 name=f"ptr2_{k}")
                nc.tensor.transpose(out=ptr2[:], in_=t4[:], identity=ident[:])
                hT = tpool.tile([P, P], f32, tag="hT", name=f"hT_{k}")
                nc.vector.tensor_copy(out=hT[:], in_=ptr2[:])
                ph2 = pmmp.tile([P, C2], f32, space="PSUM", tag="pmm",
                                name=f"ph2_{k}")
                nc.tensor.matmul(out=ph2[:], lhsT=hT[:], rhs=W2t[:],
                                 start=True, stop=True)
                nc.scalar.activation(out=h2stash[:, k * C2:(k + 1) * C2],
                                     in_=ph2[:],
                                     func=mybir.ActivationFunctionType.Copy,
                                     scale=dinvK[:, k:k + 1])
                nc.sync.dma_start(out=ag2[k * P:(k + 1) * P, :],
                                  in_=h2stash[:, k * C2:(k + 1) * C2])

            nc.gpsimd.collective_compute(
                "AllGather", mybir.AluOpType.bypass,
                replica_groups=[list(range(CORES))],
                ins=[ag2.opt()], outs=[table2.opt()])

            # ---------- conv2 ----------
            o2 = 0
            pc = 0
            for k in range(TPC):
                nch = nch_of_k[k]
                nidx = nch * P
                he = he2pool.tile([P, NCH2 * C2], f32, tag="he2",
                                  name=f"he2_{k}")
                nc.gpsimd.dma_gather(
                    out_ap=he[:, 0:nch * C2].rearrange(
                        "p (n c) -> p n c", c=C2),
                    in_ap=table2[T2_MID:, :],
                    idxs_ap=gidx2_t[:, o2 // 16:(o2 + nidx) // 16],
                    num_idxs=nidx, num_idxs_reg=nidx, elem_size=C2,
                    single_packet=False, queue_num=next_q())
                o2 += nidx
                pacc = paccp.tile([P, C2], f32, space="PSUM", tag="pacc",
                                  name=f"pacc2_{k}")
                for j in range(nch):
                    S = spool.tile([P, P], f32, tag="S2", name=f"S2_{k}_{j}")
                    nc.vector.tensor_tensor(
                        out=S[:],
                        in0=dstlf_t[:, pc + j:pc + j + 1].to_broadcast([P, P]),
                        in1=iota_t[:], op=mybir.AluOpType.is_equal)
                    nc.tensor.matmul(
                        out=pacc[:], lhsT=S[:],
                        rhs=he[:, j * C2:(j + 1) * C2],
                        start=(j == 0), stop=False)
                pc += nch
                nc.tensor.matmul(out=pacc[:], lhsT=ident[:],
                                 rhs=h2stash[:, k * C2:(k + 1) * C2],
                                 start=False, stop=False)
                nc.tensor.matmul(out=pacc[:], lhsT=rdk_t[:, k * P:(k + 1) * P],
                                 rhs=b2t[0:1, :], start=False, stop=True)
                t4 = tpool.tile([P, C2], f32, tag="u4", name=f"u4_{k}")
                nc.scalar.activation(out=t4[:], in_=pacc[:],
                                     func=mybir.ActivationFunctionType.Relu,
                                     scale=dinvK[:, k:k + 1])
                nc.sync.dma_start(out=y[k * P:(k + 1) * P, :], in_=t4[:])

    nc.compile()
    return nc


_cache = {}


def kernel(x, edge_index, emb_a, emb_b, W1, b1, W2, b2):
    in_maps, meta = prep(x, edge_index, emb_a, emb_b, W1, b1, W2, b2)
    key = (meta["nch_of_k"], meta["NPAIRS"])
    if key not in _cache:
        _cache[key] = build(meta)
    nc = _cache[key]
    res = run_bass_kernel_spmd(nc, in_maps, core_ids=list(range(CORES)))
    out = np.zeros((N, C2), dtype=np.float32)
    for c in range(CORES):
        yc = res.results[c]["y"]
        nodes = np.concatenate(
            [t * P + np.arange(P) for t in meta["core_tiles"][c]])
        valid = nodes < N
        out[nodes[valid]] = yc[valid]
    return out


# revision 11
# speedup vs baseline: 1.0952x; 1.0952x over previous
"""Self-contained GCN encoder kernel for 8 TRN2 NeuronCores (Bass/Tile).

kernel(**inputs) takes the FULL unsharded inputs (as from setup_inputs())
and returns the FULL [50000, 64] float32 output.

Strategy: stage 1 (embedding + W1) is REPLICATED on every core via a fused
host-precomputed lookup table emb_ab = emb_a@W1[:64] (+) emb_b@W1[64:128]
(one dma_gather per 8-tile group, accumulated into PSUM with an
identity-matmul, plus the numeric-feature matmul), writing the full
dinv-scaled h1 table to local DRAM -- no first AllGather, so the slow
startup CC barrier overlaps compute.  Conv aggregations shard dst-node
tiles across cores (LPT-balanced, quantile-matched slot order keeps the
SPMD stream identical); per-edge rows are fetched with dma_gather striped
over 4 SWDGE queues (4x descriptor-generation throughput) using a
mid-table base pointer and signed int16 indices (no A/B table split).
Seg-reduction is one-hot (is_equal) S-matrices x gathered rows on the
TensorEngine into PSUM; the symmetric norm is folded into table rows (src)
and the epilogue scale (dst); conv1 self-loop rows ride along as an extra
gather chunk, conv2 self-loop terms are stashed in SBUF from the conv1
epilogue.  One AllGather (h2 table) runs between the convs.
"""
import numpy as np
from concourse import bacc, mybir, tile
from concourse.bass_utils import run_bass_kernel_spmd
from concourse.masks import make_identity

P = 128
CORES = 8
N = 50000
NTILES = 392
NPAD = NTILES * P      # 50176
TPC = NTILES // CORES  # 49
NLOC = TPC * P         # 6272
C1 = 128
C2 = 64
EMB_MID = 25000
T1_MID = NPAD // 2     # 25088
T2_MID = NPAD // 2
PAD_DSTL = 30000.0
GS = 8                 # tiles per stage-1 gather op
NQ = 4                 # SWDGE queues

f32 = mybir.dt.float32
bf16 = mybir.dt.bfloat16
i16 = mybir.dt.int16


def wrap_idx(arr):
    return arr.reshape(-1, 16).T


def rup(x, m):
    return int((x + m - 1) // m * m)


def prep(x, edge_index, emb_a, emb_b, W1, b1, W2, b2):
    import ml_dtypes
    x = np.asarray(x)
    src, dst = np.asarray(edge_index[0]).astype(np.int64), \
        np.asarray(edge_index[1]).astype(np.int64)
    deg = np.bincount(dst, minlength=N).astype(np.float32) + 1.0
    dinv = np.ones(NPAD, dtype=np.float32)
    dinv[:N] = 1.0 / np.sqrt(deg)

    # ---- tile -> core assignment (LPT on edge counts) ----
    t_of_e = dst // P
    tile_cnt = np.bincount(t_of_e, minlength=NTILES)
    order = np.argsort(-tile_cnt, kind="stable")
    core_loads = np.zeros(CORES, dtype=np.int64)
    core_tiles = [[] for _ in range(CORES)]
    for t in order:
        c = int(np.argmin(core_loads))
        core_tiles[c].append(int(t))
        core_loads[c] += tile_cnt[t]
    c_of_t = np.zeros(NTILES, dtype=np.int64)
    k_of_t = np.zeros(NTILES, dtype=np.int64)
    for c in range(CORES):
        for k, t in enumerate(core_tiles[c]):
            c_of_t[t] = c
            k_of_t[t] = k

    node_ids = np.arange(NPAD)
    trow2 = c_of_t[node_ids // P] * NLOC + k_of_t[node_ids // P] * P \
        + node_ids % P

    # ---- sort edges by (core, slot) ----
    key = c_of_t[t_of_e] * TPC + k_of_t[t_of_e]
    sort = np.argsort(key, kind="stable")
    src_s = src[sort]
    trow2_s = trow2[src_s]
    dstl_s = (dst % P).astype(np.float32)[sort]
    bounds = np.searchsorted(key[sort], np.arange(CORES * TPC + 1))

    # ---- op schedule: one op per slot k; nch = max over cores ----
    nch_of_k = []
    for k in range(TPC):
        m = max(int(bounds[c * TPC + k + 1] - bounds[c * TPC + k])
                for c in range(CORES))
        nch_of_k.append(max(1, rup(m, P) // P))
    NPAIRS = sum(nch_of_k)
    NCH1 = max(nch_of_k) + 1      # +1 self chunk
    NCH2 = max(nch_of_k)
    G1COLS = sum((1 + nch) * P for nch in nch_of_k) // 16
    G2COLS = sum(nch * P for nch in nch_of_k) // 16

    # ---- per-core gather idx / dstl arrays ----
    in_maps = []
    iota = np.tile(np.arange(P, dtype=np.float32)[None, :], (P, 1))

    codes_a = np.zeros(NPAD, dtype=np.int64)
    codes_a[:N] = x[:, 0].astype(np.int64)
    codes_b = np.zeros(NPAD, dtype=np.int64)
    codes_b[:N] = x[:, 1].astype(np.int64)
    # stage-1 idx list: 49 ops x GS tiles; idx = cat_a (>=0, no trailing issue)
    eidx = np.tile(wrap_idx(codes_a.astype(np.int16)), (8, 1))

    # small gather table: emb_a@W1lo [1000, 128]; emb_b part via one-hot matmul
    emb_aw = (np.asarray(emb_a, np.float32)
              @ np.asarray(W1, np.float32)[0:64]).astype(ml_dtypes.bfloat16)
    W1Bp = (np.asarray(emb_b, np.float32)
            @ np.asarray(W1, np.float32)[64:128]).astype(ml_dtypes.bfloat16)
    xbT_rep = np.tile(codes_b.astype(np.float32)[None, :],
                      (50, 1)).astype(ml_dtypes.bfloat16)
    iotap = np.arange(P, dtype=np.float32)[:, None].astype(ml_dtypes.bfloat16)

    xT = np.zeros((8, NPAD), dtype=np.float32)
    xT[:, :N] = x[:, 2:10].T
    xT = xT.astype(ml_dtypes.bfloat16)

    dinv_all = dinv.reshape(NTILES, P).T.copy()   # [P, NTILES]

    for c in range(CORES):
        g1 = np.zeros(G1COLS * 16, dtype=np.int64)
        g2 = np.zeros(G2COLS * 16, dtype=np.int64)
        dstlm = np.full((P, NPAIRS), PAD_DSTL, dtype=np.float32)
        o1 = o2 = 0
        pc = 0
        for k in range(TPC):
            nch = nch_of_k[k]
            t = core_tiles[c][k]
            # conv1 self chunk: own tile rows
            g1[o1:o1 + P] = t * P + np.arange(P) - T1_MID
            lo, hi = bounds[c * TPC + k], bounds[c * TPC + k + 1]
            m = int(hi - lo)
            i1 = np.zeros(nch * P, dtype=np.int64)
            i2 = np.zeros(nch * P, dtype=np.int64)
            dl = np.full(nch * P, PAD_DSTL, dtype=np.float32)
            i1[:m] = src_s[lo:hi] - T1_MID
            i2[:m] = trow2_s[lo:hi] - T2_MID
            dl[:m] = dstl_s[lo:hi]
            # ensure last wrapped element (list[-1]) is >= 0 in both lists
            if i1[-1] < 0 or i2[-1] < 0:
                ok = np.where((i1 >= 0) & (i2 >= 0))[0]
                assert len(ok), "no safe trailing idx in op"
                p_ = int(ok[0])
                for arr in (i1, i2, dl):
                    arr[p_], arr[-1] = arr[-1], arr[p_]
            g1[o1 + P:o1 + P + nch * P] = i1
            g2[o2:o2 + nch * P] = i2
            for j in range(nch):
                dstlm[:, pc + j] = dl[j * P:(j + 1) * P]
            o1 += (1 + nch) * P
            o2 += nch * P
            pc += nch
        assert o1 == G1COLS * 16 and o2 == G2COLS * 16 and pc == NPAIRS

        # self-chunk trailing check: self idx can be negative only if the
        # slot's op list ends with it -- never (edge chunks follow; nch>=1)
        gidx1 = np.tile(wrap_idx(g1.astype(np.int16)), (8, 1))
        gidx2 = np.tile(wrap_idx(g2.astype(np.int16)), (8, 1))

        nodes_own = np.concatenate(
            [t * P + np.arange(P) for t in core_tiles[c]])
        dinvk = dinv[nodes_own].reshape(TPC, P).T.copy()

        rdk = (1.0 / dinvk).reshape(1, -1, order="F").astype(np.float32)

        in_maps.append({
            "emb_aw": emb_aw, "xbT_rep": xbT_rep,
            "iotap": iotap,
            "wcomb": np.concatenate([
                W1Bp.astype(np.float32),
                np.zeros((14, C1), np.float32),
                np.asarray(W1, np.float32)[128:136]]).astype(ml_dtypes.bfloat16),
            "xT": xT,
            "eidx": eidx.copy(),
            "gidx1": gidx1,
            "gidx2": gidx2,
            "dstlm": dstlm.astype(ml_dtypes.bfloat16), "dstlf": dstlm,
            "dinv_all": dinv_all,
            "dinvk": dinvk,
            "W2": np.asarray(W2, dtype=np.float32),
            "b1f": np.tile(np.asarray(b1, np.float32)[None, :], (P, 1)),
            "b2f": np.tile(np.asarray(b2, np.float32)[None, :], (P, 1)),
            "iota": iota, "iotab": iota.astype(ml_dtypes.bfloat16), "rdk": rdk,
        })

    meta = {"nch_of_k": tuple(nch_of_k), "NPAIRS": NPAIRS, "NCH1": NCH1,
            "NCH2": NCH2, "G1COLS": G1COLS, "G2COLS": G2COLS,
            "core_tiles": core_tiles}
    return in_maps, meta


def build(meta):
    nch_of_k = meta["nch_of_k"]
    NPAIRS = meta["NPAIRS"]
    NCH1 = meta["NCH1"]
    NCH2 = meta["NCH2"]
    G1COLS = meta["G1COLS"]
    G2COLS = meta["G2COLS"]
    ECOLS = NTILES * P // 16

    nc = bacc.Bacc("TRN2", target_bir_lowering=False, debug=False,
                   num_devices=CORES, num_swdge_queues=NQ)
    emb_aw = nc.dram_tensor("emb_aw", [1000, C1], bf16, kind="ExternalInput")
    wcomb = nc.dram_tensor("wcomb", [72, C1], bf16, kind="ExternalInput")
    xbT_rep = nc.dram_tensor("xbT_rep", [50, NPAD], bf16, kind="ExternalInput")
    iotap = nc.dram_tensor("iotap", [P, 1], bf16, kind="ExternalInput")
    xT = nc.dram_tensor("xT", [8, NPAD], bf16, kind="ExternalInput")
    eidx = nc.dram_tensor("eidx", [P, ECOLS], i16, kind="ExternalInput")
    gidx1 = nc.dram_tensor("gidx1", [P, G1COLS], i16, kind="ExternalInput")
    gidx2 = nc.dram_tensor("gidx2", [P, G2COLS], i16, kind="ExternalInput")
    dstlm = nc.dram_tensor("dstlm", [P, NPAIRS], bf16, kind="ExternalInput")
    iotab = nc.dram_tensor("iotab", [P, P], bf16, kind="ExternalInput")
    dstlf = nc.dram_tensor("dstlf", [P, NPAIRS], f32, kind="ExternalInput")
    rdk = nc.dram_tensor("rdk", [1, NLOC], f32, kind="ExternalInput")
    dinv_all = nc.dram_tensor("dinv_all", [P, NTILES], f32, kind="ExternalInput")
    dinvk = nc.dram_tensor("dinvk", [P, TPC], f32, kind="ExternalInput")
    W2 = nc.dram_tensor("W2", [C1, C2], f32, kind="ExternalInput")
    b1f = nc.dram_tensor("b1f", [P, C1], f32, kind="ExternalInput")
    b2f = nc.dram_tensor("b2f", [P, C2], f32, kind="ExternalInput")
    iota = nc.dram_tensor("iota", [P, P], f32, kind="ExternalInput")
    y = nc.dram_tensor("y", [NLOC, C2], f32, kind="ExternalOutput")

    with tile.TileContext(nc) as tc:
        with tc.tile_pool(name="const", bufs=1) as cpool, \
             tc.tile_pool(name="meta", bufs=1) as mpool, \
             tc.tile_pool(name="ge", bufs=10) as gepool, \
             tc.tile_pool(name="xt", bufs=2) as xtpool, \
             tc.tile_pool(name="he1", bufs=7) as he1pool, \
             tc.tile_pool(name="he2", bufs=7) as he2pool, \
             tc.tile_pool(name="sel", bufs=4) as spool, \
             tc.tile_pool(name="epi", bufs=3) as tpool, \
             tc.tile_pool(name="stash", bufs=1) as stpool, \
             tc.tile_pool(name="ptr", bufs=1, space="PSUM") as ptrp, \
             tc.tile_pool(name="pmm", bufs=2, space="PSUM") as pmmp, \
             tc.tile_pool(name="pacc", bufs=5, space="PSUM") as paccp, \
             tc.tile_pool(name="dram", bufs=1, space="DRAM") as dram:

            # ---------- constants ----------
            ident = cpool.tile([P, P], f32, tag="ident")
            make_identity(nc, ident[:])
            identb = cpool.tile([P, P], bf16, tag="identb")
            nc.vector.tensor_copy(out=identb[:], in_=ident[:])
            iota_t = cpool.tile([P, P], f32, tag="iota")
            nc.sync.dma_start(out=iota_t[:], in_=iota[:])
            iotab_t = cpool.tile([P, P], bf16, tag="iotab")
            nc.sync.dma_start(out=iotab_t[:], in_=iotab[:])
            iotap_t = cpool.tile([P, 1], bf16, tag="iotap")
            nc.sync.dma_start(out=iotap_t[:], in_=iotap[:])
            rdk_t = cpool.tile([1, NLOC], f32, tag="rdk")
            nc.sync.dma_start(out=rdk_t[:], in_=rdk[:])
            wcomb_t = cpool.tile([72, C1], bf16, tag="wcomb")
            nc.sync.dma_start(out=wcomb_t[:], in_=wcomb[:])
            W2t = cpool.tile([C1, C2], f32, tag="w2")
            nc.sync.dma_start(out=W2t[:], in_=W2[:])
            b1t = cpool.tile([P, C1], f32, tag="b1")
            nc.sync.dma_start(out=b1t[:], in_=b1f[:])
            b2t = cpool.tile([P, C2], f32, tag="b2")
            nc.sync.dma_start(out=b2t[:], in_=b2f[:])
            dinvA = cpool.tile([P, NTILES], f32, tag="dinvA")
            nc.sync.dma_start(out=dinvA[:], in_=dinv_all[:])
            dinvK = cpool.tile([P, TPC], f32, tag="dinvK")
            nc.sync.dma_start(out=dinvK[:], in_=dinvk[:])
            eidx_t = mpool.tile([P, ECOLS], i16, tag="eidx")
            nc.sync.dma_start(out=eidx_t[:], in_=eidx[:])
            h2stash = stpool.tile([P, TPC * C2], f32, tag="h2stash")

            table1 = dram.tile([NPAD, C1], bf16, tag="table1")
            ag2 = dram.tile([NLOC, C2], f32, tag="ag2")
            table2 = dram.tile([NPAD, C2], f32, tag="table2")

            gq = [0]

            def next_q():
                q = gq[0] % NQ
                gq[0] += 1
                return q

            # ---------- stage 1 (replicated): build full h1 table ----------
            for e in range(NTILES // GS):
                nidx = GS * P
                ge = gepool.tile([P, GS * P], bf16, tag="ge", name=f"ge_{e}")
                nc.gpsimd.dma_gather(
                    out_ap=ge[:].rearrange("p (n c) -> p n c", c=C1),
                    in_ap=emb_aw[:],
                    idxs_ap=eidx_t[:, e * nidx // 16:(e + 1) * nidx // 16],
                    num_idxs=nidx, num_idxs_reg=nidx, elem_size=C1,
                    single_packet=False, queue_num=next_q())
                xb_c = xtpool.tile([50, GS * P], bf16, tag="xb", name=f"xb_{e}")
                nc.sync.dma_start(out=xb_c[:],
                                  in_=xbT_rep[:, e * GS * P:(e + 1) * GS * P])
                comb = gepool.tile([72, GS * P], bf16, tag="ob", name=f"ob_{e}")
                nc.sync.dma_start(out=comb[64:72, :],
                                  in_=xT[:, e * GS * P:(e + 1) * GS * P])
                nc.vector.memset(comb[32:64, :], 0.0)
                nc.vector.tensor_tensor(
                    out=comb[0:50, :], in0=xb_c[:],
                    in1=iotap_t[0:50, 0:1].to_broadcast([50, GS * P]),
                    op=mybir.AluOpType.is_equal)
                h1st = xtpool.tile([P, GS * C1], bf16, tag="h1st",
                                   name=f"h1st_{e}")
                for half in range(GS // 4):
                    w = e * (GS // 4) + half
                    php = pmmp if w % 2 == 0 else paccp
                    wide = php.tile([P, 4 * C1], f32, space="PSUM",
                                    tag="pmm" if w % 2 == 0 else "pacc",
                                    name=f"wide_{w}")
                    nc.tensor.matmul(out=wide[:], lhsT=identb[:],
                                     rhs=ge[:, half * 4 * C1:(half + 1) * 4 * C1],
                                     start=True, stop=False)
                    for jj in range(4):
                        j = half * 4 + jj
                        t = e * GS + j
                        nc.tensor.matmul(
                            out=wide[:, jj * C1:(jj + 1) * C1],
                            lhsT=comb[:, j * P:(j + 1) * P],
                            rhs=wcomb_t[:], start=False, stop=True)
                    for jj in range(4):
                        j = half * 4 + jj
                        t = e * GS + j
                        if t % 2 == 0:
                            nc.scalar.activation(
                                out=h1st[:, j * C1:(j + 1) * C1],
                                in_=wide[:, jj * C1:(jj + 1) * C1],
                                func=mybir.ActivationFunctionType.Copy,
                                scale=dinvA[:, t:t + 1])
                        else:
                            nc.vector.tensor_tensor(
                                out=h1st[:, j * C1:(j + 1) * C1],
                                in0=wide[:, jj * C1:(jj + 1) * C1],
                                in1=dinvA[:, t:t + 1].to_broadcast([P, C1]),
                                op=mybir.AluOpType.mult)
                nc.sync.dma_start(
                    out=table1[e * GS * P:(e + 1) * GS * P, :].rearrange(
                        "(n p) c -> p n c", p=P),
                    in_=h1st[:].rearrange("p (n c) -> p n c", c=C1))

            # conv metadata loads (overlap stage-1)
            gidx1_t = mpool.tile([P, G1COLS], i16, tag="gidx1")
            nc.sync.dma_start(out=gidx1_t[:], in_=gidx1[:])
            gidx2_t = mpool.tile([P, G2COLS], i16, tag="gidx2")
            nc.sync.dma_start(out=gidx2_t[:], in_=gidx2[:])
            dstl_t = mpool.tile([P, NPAIRS], bf16, tag="dstl")
            nc.sync.dma_start(out=dstl_t[:], in_=dstlm[:])
            dstlf_t = mpool.tile([P, NPAIRS], f32, tag="dstlf")
            nc.sync.dma_start(out=dstlf_t[:], in_=dstlf[:])

            tc.strict_bb_all_engine_barrier()

            # ---------- conv1 ----------
            o1 = 0
            pc = 0
            for k in range(TPC):
                nch = nch_of_k[k]
                nidx = (1 + nch) * P
                he = he1pool.tile([P, NCH1 * C1], bf16, tag="he1",
                                  name=f"he1_{k}")
                nc.gpsimd.dma_gather(
                    out_ap=he[:, 0:(1 + nch) * C1].rearrange(
                        "p (n c) -> p n c", c=C1),
                    in_ap=table1[T1_MID:, :],
                    idxs_ap=gidx1_t[:, o1 // 16:(o1 + nidx) // 16],
                    num_idxs=nidx, num_idxs_reg=nidx, elem_size=C1,
                    single_packet=False, queue_num=next_q())
                o1 += nidx
                pacc = paccp.tile([P, C1], f32, space="PSUM", tag="pacc",
                                  name=f"pacc1_{k}")
                for j in range(nch):
                    S = spool.tile([P, P], bf16, tag="S1", name=f"S1_{k}_{j}")
                    nc.vector.tensor_tensor(
                        out=S[:],
                        in0=dstl_t[:, pc + j:pc + j + 1].to_broadcast([P, P]),
                        in1=iotab_t[:], op=mybir.AluOpType.is_equal)
                    nc.tensor.matmul(
                        out=pacc[:], lhsT=S[:],
                        rhs=he[:, (1 + j) * C1:(2 + j) * C1],
                        start=(j == 0), stop=False)
                pc += nch
                # pacc += self rows; pacc += b1/dinv (so relu(dinv*pacc) is exact)
                nc.tensor.matmul(out=pacc[:], lhsT=identb[:], rhs=he[:, 0:C1],
                                 start=False, stop=False)
                nc.tensor.matmul(out=pacc[:], lhsT=rdk_t[:, k * P:(k + 1) * P],
                                 rhs=b1t[0:1, :], start=False, stop=True)
                t4 = tpool.tile([P, C1], f32, tag="t4", name=f"t4_{k}")
                nc.scalar.activation(out=t4[:], in_=pacc[:],
                                     func=mybir.ActivationFunctionType.Relu,
                                     scale=dinvK[:, k:k + 1])
                # h2 = (t4 @ W2) * dinv
                ptr2 = ptrp.tile([P, P], f32, space="PSUM", tag="ptr",
                                 name=f"ptr2_{k}")
                nc.tensor.transpose(out=ptr2[:], in_=t4[:], identity=ident[:])
                hT = tpool.tile([P, P], f32, tag="hT", name=f"hT_{k}")
                nc.vector.tensor_copy(out=hT[:], in_=ptr2[:])
                ph2 = pmmp.tile([P, C2], f32, space="PSUM", tag="pmm",
                                name=f"ph2_{k}")
                nc.tensor.matmul(out=ph2[:], lhsT=hT[:], rhs=W2t[:],
                                 start=True, stop=True)
                nc.scalar.activation(out=h2stash[:, k * C2:(k + 1) * C2],
                                     in_=ph2[:],
                                     func=mybir.ActivationFunctionType.Copy,
                                     scale=dinvK[:, k:k + 1])
                nc.sync.dma_start(out=ag2[k * P:(k + 1) * P, :],
                                  in_=h2stash[:, k * C2:(k + 1) * C2])

            nc.gpsimd.collective_compute(
                "AllGather", mybir.AluOpType.bypass,
                replica_groups=[list(range(CORES))],
                ins=[ag2.opt()], outs=[table2.opt()])

            # ---------- conv2 ----------
            o2 = 0
            pc = 0
            for k in range(TPC):
                nch = nch_of_k[k]
                nidx = nch * P
                he = he2pool.tile([P, NCH2 * C2], f32, tag="he2",
                                  name=f"he2_{k}")
                nc.gpsimd.dma_gather(
                    out_ap=he[:, 0:nch * C2].rearrange(
                        "p (n c) -> p n c", c=C2),
                    in_ap=table2[T2_MID:, :],
                    idxs_ap=gidx2_t[:, o2 // 16:(o2 + nidx) // 16],
                    num_idxs=nidx, num_idxs_reg=nidx, elem_size=C2,
                    single_packet=False, queue_num=next_q())
                o2 += nidx
                pacc = paccp.tile([P, C2], f32, space="PSUM", tag="pacc",
                                  name=f"pacc2_{k}")
                for j in range(nch):
                    S = spool.tile([P, P], f32, tag="S2", name=f"S2_{k}_{j}")
                    nc.vector.tensor_tensor(
                        out=S[:],
                        in0=dstlf_t[:, pc + j:pc + j + 1].to_broadcast([P, P]),
                        in1=iota_t[:], op=mybir.AluOpType.is_equal)
                    nc.tensor.matmul(
                        out=pacc[:], lhsT=S[:],
                        rhs=he[:, j * C2:(j + 1) * C2],
                        start=(j == 0), stop=False)
                pc += nch
                nc.tensor.matmul(out=pacc[:], lhsT=ident[:],
                                 rhs=h2stash[:, k * C2:(k + 1) * C2],
                                 start=False, stop=False)
                nc.tensor.matmul(out=pacc[:], lhsT=rdk_t[:, k * P:(k + 1) * P],
                                 rhs=b2t[0:1, :], start=False, stop=True)
                t4 = tpool.tile([P, C2], f32, tag="u4", name=f"u4_{k}")
                nc.scalar.activation(out=t4[:], in_=pacc[:],
                                     func=mybir.ActivationFunctionType.Relu,
                                     scale=dinvK[:, k:k + 1])
                nc.sync.dma_start(out=y[k * P:(k + 1) * P, :], in_=t4[:])

    nc.compile()
    return nc


_cache = {}


def kernel(x, edge_index, emb_a, emb_b, W1, b1, W2, b2):
    in_maps, meta = prep(x, edge_index, emb_a, emb_b, W1, b1, W2, b2)
    key = (meta["nch_of_k"], meta["NPAIRS"])
    if key not in _cache:
        _cache[key] = build(meta)
    nc = _cache[key]
    res = run_bass_kernel_spmd(nc, in_maps, core_ids=list(range(CORES)))
    out = np.zeros((N, C2), dtype=np.float32)
    for c in range(CORES):
        yc = res.results[c]["y"]
        nodes = np.concatenate(
            [t * P + np.arange(P) for t in meta["core_tiles"][c]])
        valid = nodes < N
        out[nodes[valid]] = yc[valid]
    return out


# revision 12
# speedup vs baseline: 1.1138x; 1.0169x over previous
"""Self-contained GCN encoder kernel for 8 TRN2 NeuronCores (Bass/Tile).

kernel(**inputs) takes the FULL unsharded inputs (as from setup_inputs())
and returns the FULL [50000, 64] float32 output.

Strategy: stage 1 (embedding + W1) is REPLICATED on every core via a fused
host-precomputed lookup table emb_ab = emb_a@W1[:64] (+) emb_b@W1[64:128]
(one dma_gather per 8-tile group, accumulated into PSUM with an
identity-matmul, plus the numeric-feature matmul), writing the full
dinv-scaled h1 table to local DRAM -- no first AllGather, so the slow
startup CC barrier overlaps compute.  Conv aggregations shard dst-node
tiles across cores (LPT-balanced, quantile-matched slot order keeps the
SPMD stream identical); per-edge rows are fetched with dma_gather striped
over 4 SWDGE queues (4x descriptor-generation throughput) using a
mid-table base pointer and signed int16 indices (no A/B table split).
Seg-reduction is one-hot (is_equal) S-matrices x gathered rows on the
TensorEngine into PSUM; the symmetric norm is folded into table rows (src)
and the epilogue scale (dst); conv1 self-loop rows ride along as an extra
gather chunk, conv2 self-loop terms are stashed in SBUF from the conv1
epilogue.  One AllGather (h2 table) runs between the convs.
"""
import numpy as np
from concourse import bacc, mybir, tile
from concourse.bass_utils import run_bass_kernel_spmd
from concourse.masks import make_identity

P = 128
CORES = 8
N = 50000
NTILES = 392
NPAD = NTILES * P      # 50176
TPC = NTILES // CORES  # 49
NLOC = TPC * P         # 6272
C1 = 128
C2 = 64
EMB_MID = 25000
T1_MID = NPAD // 2     # 25088
T2_MID = NPAD // 2
PAD_DSTL = 30000.0
GS = 8                 # tiles per stage-1 gather op
NQ = 4                 # SWDGE queues

f32 = mybir.dt.float32
bf16 = mybir.dt.bfloat16
i16 = mybir.dt.int16


def wrap_idx(arr):
    return arr.reshape(-1, 16).T


def rup(x, m):
    return int((x + m - 1) // m * m)


def prep(x, edge_index, emb_a, emb_b, W1, b1, W2, b2):
    import ml_dtypes
    x = np.asarray(x)
    src, dst = np.asarray(edge_index[0]).astype(np.int64), \
        np.asarray(edge_index[1]).astype(np.int64)
    deg = np.bincount(dst, minlength=N).astype(np.float32) + 1.0
    dinv = np.ones(NPAD, dtype=np.float32)
    dinv[:N] = 1.0 / np.sqrt(deg)

    # ---- tile -> core assignment (LPT on edge counts) ----
    t_of_e = dst // P
    tile_cnt = np.bincount(t_of_e, minlength=NTILES)
    order = np.argsort(-tile_cnt, kind="stable")
    core_loads = np.zeros(CORES, dtype=np.int64)
    core_tiles = [[] for _ in range(CORES)]
    for t in order:
        c = int(np.argmin(core_loads))
        core_tiles[c].append(int(t))
        core_loads[c] += tile_cnt[t]
    c_of_t = np.zeros(NTILES, dtype=np.int64)
    k_of_t = np.zeros(NTILES, dtype=np.int64)
    for c in range(CORES):
        for k, t in enumerate(core_tiles[c]):
            c_of_t[t] = c
            k_of_t[t] = k

    node_ids = np.arange(NPAD)
    trow2 = c_of_t[node_ids // P] * NLOC + k_of_t[node_ids // P] * P \
        + node_ids % P

    # ---- sort edges by (core, slot) ----
    key = c_of_t[t_of_e] * TPC + k_of_t[t_of_e]
    sort = np.argsort(key, kind="stable")
    src_s = src[sort]
    trow2_s = trow2[src_s]
    dstl_s = (dst % P).astype(np.float32)[sort]
    bounds = np.searchsorted(key[sort], np.arange(CORES * TPC + 1))

    # ---- op schedule: one op per slot k; nch = max over cores ----
    nch_of_k = []
    for k in range(TPC):
        m = max(int(bounds[c * TPC + k + 1] - bounds[c * TPC + k])
                for c in range(CORES))
        nch_of_k.append(max(1, rup(m, P) // P))
    NPAIRS = sum(nch_of_k)
    NCH1 = max(nch_of_k) + 1      # +1 self chunk
    NCH2 = max(nch_of_k)
    G1COLS = sum((1 + nch) * P for nch in nch_of_k) // 16
    G2COLS = sum(nch * P for nch in nch_of_k) // 16

    # ---- per-core gather idx / dstl arrays ----
    in_maps = []
    iota = np.tile(np.arange(P, dtype=np.float32)[None, :], (P, 1))

    codes_a = np.zeros(NPAD, dtype=np.int64)
    codes_a[:N] = x[:, 0].astype(np.int64)
    codes_b = np.zeros(NPAD, dtype=np.int64)
    codes_b[:N] = x[:, 1].astype(np.int64)
    # stage-1 idx list: 49 ops x GS tiles; idx = cat_a (>=0, no trailing issue)
    eidx = np.tile(wrap_idx(codes_a.astype(np.int16)), (8, 1))

    # small gather table: emb_a@W1lo [1000, 128]; emb_b part via one-hot matmul
    emb_aw = (np.asarray(emb_a, np.float32)
              @ np.asarray(W1, np.float32)[0:64]).astype(ml_dtypes.bfloat16)
    W1Bp = (np.asarray(emb_b, np.float32)
            @ np.asarray(W1, np.float32)[64:128]).astype(ml_dtypes.bfloat16)
    xbT_rep = np.tile(codes_b.astype(np.float32)[None, :],
                      (50, 1)).astype(ml_dtypes.bfloat16)
    iotap = np.arange(P, dtype=np.float32)[:, None].astype(ml_dtypes.bfloat16)

    xT = np.zeros((8, NPAD), dtype=np.float32)
    xT[:, :N] = x[:, 2:10].T
    xT = xT.astype(ml_dtypes.bfloat16)

    dinv_all = dinv.reshape(NTILES, P).T.copy()   # [P, NTILES]

    for c in range(CORES):
        g1 = np.zeros(G1COLS * 16, dtype=np.int64)
        g2 = np.zeros(G2COLS * 16, dtype=np.int64)
        dstlm = np.full((P, NPAIRS), PAD_DSTL, dtype=np.float32)
        o1 = o2 = 0
        pc = 0
        for k in range(TPC):
            nch = nch_of_k[k]
            t = core_tiles[c][k]
            # conv1 self chunk: own tile rows
            g1[o1:o1 + P] = t * P + np.arange(P) - T1_MID
            lo, hi = bounds[c * TPC + k], bounds[c * TPC + k + 1]
            m = int(hi - lo)
            i1 = np.zeros(nch * P, dtype=np.int64)
            i2 = np.zeros(nch * P, dtype=np.int64)
            dl = np.full(nch * P, PAD_DSTL, dtype=np.float32)
            # sort edges by src table row for HBM row-buffer locality
            so1 = np.argsort(src_s[lo:hi], kind="stable")
            i1[:m] = (src_s[lo:hi] - T1_MID)[so1]
            dl[:m] = dstl_s[lo:hi][so1]
            i2[:m] = (trow2_s[lo:hi] - T2_MID)[so1]
            # ensure last wrapped element (list[-1]) is >= 0 in both lists
            if i1[-1] < 0 or i2[-1] < 0:
                ok = np.where((i1 >= 0) & (i2 >= 0))[0]
                assert len(ok), "no safe trailing idx in op"
                p_ = int(ok[0])
                for arr in (i1, i2, dl):
                    arr[p_], arr[-1] = arr[-1], arr[p_]
            g1[o1 + P:o1 + P + nch * P] = i1
            g2[o2:o2 + nch * P] = i2
            for j in range(nch):
                dstlm[:, pc + j] = dl[j * P:(j + 1) * P]
            o1 += (1 + nch) * P
            o2 += nch * P
            pc += nch
        assert o1 == G1COLS * 16 and o2 == G2COLS * 16 and pc == NPAIRS

        # self-chunk trailing check: self idx can be negative only if the
        # slot's op list ends with it -- never (edge chunks follow; nch>=1)
        gidx1 = np.tile(wrap_idx(g1.astype(np.int16)), (8, 1))
        gidx2 = np.tile(wrap_idx(g2.astype(np.int16)), (8, 1))

        nodes_own = np.concatenate(
            [t * P + np.arange(P) for t in core_tiles[c]])
        dinvk = dinv[nodes_own].reshape(TPC, P).T.copy()

        rdk = (1.0 / dinvk).reshape(1, -1, order="F").astype(np.float32)

        in_maps.append({
            "emb_aw": emb_aw, "xbT_rep": xbT_rep,
            "iotap": iotap,
            "wcomb": np.concatenate([
                W1Bp.astype(np.float32),
                np.zeros((14, C1), np.float32),
                np.asarray(W1, np.float32)[128:136]]).astype(ml_dtypes.bfloat16),
            "xT": xT,
            "eidx": eidx.copy(),
            "gidx1": gidx1,
            "gidx2": gidx2,
            "dstlm": dstlm.astype(ml_dtypes.bfloat16), "dstlf": dstlm,
            "dinv_all": dinv_all,
            "dinvk": dinvk,
            "W2": np.asarray(W2, dtype=np.float32),
            "b1f": np.tile(np.asarray(b1, np.float32)[None, :], (P, 1)),
            "b2f": np.tile(np.asarray(b2, np.float32)[None, :], (P, 1)),
            "iota": iota, "iotab": iota.astype(ml_dtypes.bfloat16), "rdk": rdk,
        })

    meta = {"nch_of_k": tuple(nch_of_k), "NPAIRS": NPAIRS, "NCH1": NCH1,
            "NCH2": NCH2, "G1COLS": G1COLS, "G2COLS": G2COLS,
            "core_tiles": core_tiles}
    return in_maps, meta


def build(meta):
    nch_of_k = meta["nch_of_k"]
    NPAIRS = meta["NPAIRS"]
    NCH1 = meta["NCH1"]
    NCH2 = meta["NCH2"]
    G1COLS = meta["G1COLS"]
    G2COLS = meta["G2COLS"]
    ECOLS = NTILES * P // 16

    nc = bacc.Bacc("TRN2", target_bir_lowering=False, debug=False,
                   num_devices=CORES, num_swdge_queues=NQ)
    emb_aw = nc.dram_tensor("emb_aw", [1000, C1], bf16, kind="ExternalInput")
    wcomb = nc.dram_tensor("wcomb", [72, C1], bf16, kind="ExternalInput")
    xbT_rep = nc.dram_tensor("xbT_rep", [50, NPAD], bf16, kind="ExternalInput")
    iotap = nc.dram_tensor("iotap", [P, 1], bf16, kind="ExternalInput")
    xT = nc.dram_tensor("xT", [8, NPAD], bf16, kind="ExternalInput")
    eidx = nc.dram_tensor("eidx", [P, ECOLS], i16, kind="ExternalInput")
    gidx1 = nc.dram_tensor("gidx1", [P, G1COLS], i16, kind="ExternalInput")
    gidx2 = nc.dram_tensor("gidx2", [P, G2COLS], i16, kind="ExternalInput")
    dstlm = nc.dram_tensor("dstlm", [P, NPAIRS], bf16, kind="ExternalInput")
    iotab = nc.dram_tensor("iotab", [P, P], bf16, kind="ExternalInput")
    dstlf = nc.dram_tensor("dstlf", [P, NPAIRS], f32, kind="ExternalInput")
    rdk = nc.dram_tensor("rdk", [1, NLOC], f32, kind="ExternalInput")
    dinv_all = nc.dram_tensor("dinv_all", [P, NTILES], f32, kind="ExternalInput")
    dinvk = nc.dram_tensor("dinvk", [P, TPC], f32, kind="ExternalInput")
    W2 = nc.dram_tensor("W2", [C1, C2], f32, kind="ExternalInput")
    b1f = nc.dram_tensor("b1f", [P, C1], f32, kind="ExternalInput")
    b2f = nc.dram_tensor("b2f", [P, C2], f32, kind="ExternalInput")
    iota = nc.dram_tensor("iota", [P, P], f32, kind="ExternalInput")
    y = nc.dram_tensor("y", [NLOC, C2], f32, kind="ExternalOutput")

    with tile.TileContext(nc) as tc:
        with tc.tile_pool(name="const", bufs=1) as cpool, \
             tc.tile_pool(name="meta", bufs=1) as mpool, \
             tc.tile_pool(name="ge", bufs=10) as gepool, \
             tc.tile_pool(name="xt", bufs=2) as xtpool, \
             tc.tile_pool(name="he1", bufs=7) as he1pool, \
             tc.tile_pool(name="he2", bufs=7) as he2pool, \
             tc.tile_pool(name="sel", bufs=4) as spool, \
             tc.tile_pool(name="epi", bufs=3) as tpool, \
             tc.tile_pool(name="stash", bufs=1) as stpool, \
             tc.tile_pool(name="ptr", bufs=1, space="PSUM") as ptrp, \
             tc.tile_pool(name="pmm", bufs=2, space="PSUM") as pmmp, \
             tc.tile_pool(name="pacc", bufs=5, space="PSUM") as paccp, \
             tc.tile_pool(name="dram", bufs=1, space="DRAM") as dram:

            # ---------- constants ----------
            ident = cpool.tile([P, P], f32, tag="ident")
            make_identity(nc, ident[:])
            identb = cpool.tile([P, P], bf16, tag="identb")
            nc.vector.tensor_copy(out=identb[:], in_=ident[:])
            iota_t = cpool.tile([P, P], f32, tag="iota")
            nc.sync.dma_start(out=iota_t[:], in_=iota[:])
            iotab_t = cpool.tile([P, P], bf16, tag="iotab")
            nc.sync.dma_start(out=iotab_t[:], in_=iotab[:])
            iotap_t = cpool.tile([P, 1], bf16, tag="iotap")
            nc.sync.dma_start(out=iotap_t[:], in_=iotap[:])
            rdk_t = cpool.tile([1, NLOC], f32, tag="rdk")
            nc.sync.dma_start(out=rdk_t[:], in_=rdk[:])
            wcomb_t = cpool.tile([72, C1], bf16, tag="wcomb")
            nc.sync.dma_start(out=wcomb_t[:], in_=wcomb[:])
            W2t = cpool.tile([C1, C2], f32, tag="w2")
            nc.sync.dma_start(out=W2t[:], in_=W2[:])
            b1t = cpool.tile([P, C1], f32, tag="b1")
            nc.sync.dma_start(out=b1t[:], in_=b1f[:])
            b2t = cpool.tile([P, C2], f32, tag="b2")
            nc.sync.dma_start(out=b2t[:], in_=b2f[:])
            dinvA = cpool.tile([P, NTILES], f32, tag="dinvA")
            nc.sync.dma_start(out=dinvA[:], in_=dinv_all[:])
            dinvK = cpool.tile([P, TPC], f32, tag="dinvK")
            nc.sync.dma_start(out=dinvK[:], in_=dinvk[:])
            eidx_t = mpool.tile([P, ECOLS], i16, tag="eidx")
            nc.sync.dma_start(out=eidx_t[:], in_=eidx[:])
            h2stash = stpool.tile([P, TPC * C2], f32, tag="h2stash")

            table1 = dram.tile([NPAD, C1], bf16, tag="table1")
            ag2 = dram.tile([NLOC, C2], f32, tag="ag2")
            table2 = dram.tile([NPAD, C2], f32, tag="table2")

            gq = [0]

            def next_q():
                q = gq[0] % NQ
                gq[0] += 1
                return q

            # ---------- stage 1 (replicated): build full h1 table ----------
            for e in range(NTILES // GS):
                nidx = GS * P
                ge = gepool.tile([P, GS * P], bf16, tag="ge", name=f"ge_{e}")
                nc.gpsimd.dma_gather(
                    out_ap=ge[:].rearrange("p (n c) -> p n c", c=C1),
                    in_ap=emb_aw[:],
                    idxs_ap=eidx_t[:, e * nidx // 16:(e + 1) * nidx // 16],
                    num_idxs=nidx, num_idxs_reg=nidx, elem_size=C1,
                    single_packet=False, queue_num=next_q())
                xb_c = xtpool.tile([50, GS * P], bf16, tag="xb", name=f"xb_{e}")
                nc.sync.dma_start(out=xb_c[:],
                                  in_=xbT_rep[:, e * GS * P:(e + 1) * GS * P])
                comb = gepool.tile([72, GS * P], bf16, tag="ob", name=f"ob_{e}")
                nc.sync.dma_start(out=comb[64:72, :],
                                  in_=xT[:, e * GS * P:(e + 1) * GS * P])
                nc.vector.memset(comb[32:64, :], 0.0)
                nc.vector.tensor_tensor(
                    out=comb[0:50, :], in0=xb_c[:],
                    in1=iotap_t[0:50, 0:1].to_broadcast([50, GS * P]),
                    op=mybir.AluOpType.is_equal)
                h1st = xtpool.tile([P, GS * C1], bf16, tag="h1st",
                                   name=f"h1st_{e}")
                for half in range(GS // 4):
                    w = e * (GS // 4) + half
                    php = pmmp if w % 2 == 0 else paccp
                    wide = php.tile([P, 4 * C1], f32, space="PSUM",
                                    tag="pmm" if w % 2 == 0 else "pacc",
                                    name=f"wide_{w}")
                    nc.tensor.matmul(out=wide[:], lhsT=identb[:],
                                     rhs=ge[:, half * 4 * C1:(half + 1) * 4 * C1],
                                     start=True, stop=False)
                    for jj in range(4):
                        j = half * 4 + jj
                        t = e * GS + j
                        nc.tensor.matmul(
                            out=wide[:, jj * C1:(jj + 1) * C1],
                            lhsT=comb[:, j * P:(j + 1) * P],
                            rhs=wcomb_t[:], start=False, stop=True)
                    for jj in range(4):
                        j = half * 4 + jj
                        t = e * GS + j
                        if t % 2 == 0:
                            nc.scalar.activation(
                                out=h1st[:, j * C1:(j + 1) * C1],
                                in_=wide[:, jj * C1:(jj + 1) * C1],
                                func=mybir.ActivationFunctionType.Copy,
                                scale=dinvA[:, t:t + 1])
                        else:
                            nc.vector.tensor_tensor(
                                out=h1st[:, j * C1:(j + 1) * C1],
                                in0=wide[:, jj * C1:(jj + 1) * C1],
                                in1=dinvA[:, t:t + 1].to_broadcast([P, C1]),
                                op=mybir.AluOpType.mult)
                nc.sync.dma_start(
                    out=table1[e * GS * P:(e + 1) * GS * P, :].rearrange(
                        "(n p) c -> p n c", p=P),
                    in_=h1st[:].rearrange("p (n c) -> p n c", c=C1))

            # conv metadata loads (overlap stage-1)
            gidx1_t = mpool.tile([P, G1COLS], i16, tag="gidx1")
            nc.sync.dma_start(out=gidx1_t[:], in_=gidx1[:])
            gidx2_t = mpool.tile([P, G2COLS], i16, tag="gidx2")
            nc.sync.dma_start(out=gidx2_t[:], in_=gidx2[:])
            dstl_t = mpool.tile([P, NPAIRS], bf16, tag="dstl")
            nc.sync.dma_start(out=dstl_t[:], in_=dstlm[:])
            dstlf_t = mpool.tile([P, NPAIRS], f32, tag="dstlf")
            nc.sync.dma_start(out=dstlf_t[:], in_=dstlf[:])

            tc.strict_bb_all_engine_barrier()

            # ---------- conv1 ----------
            o1 = 0
            pc = 0
            for k in range(TPC):
                nch = nch_of_k[k]
                nidx = (1 + nch) * P
                he = he1pool.tile([P, NCH1 * C1], bf16, tag="he1",
                                  name=f"he1_{k}")
                nc.gpsimd.dma_gather(
                    out_ap=he[:, 0:(1 + nch) * C1].rearrange(
                        "p (n c) -> p n c", c=C1),
                    in_ap=table1[T1_MID:, :],
                    idxs_ap=gidx1_t[:, o1 // 16:(o1 + nidx) // 16],
                    num_idxs=nidx, num_idxs_reg=nidx, elem_size=C1,
                    single_packet=False, queue_num=next_q())
                o1 += nidx
                pacc = paccp.tile([P, C1], f32, space="PSUM", tag="pacc",
                                  name=f"pacc1_{k}")
                for j in range(nch):
                    S = spool.tile([P, P], bf16, tag="S1", name=f"S1_{k}_{j}")
                    nc.vector.tensor_tensor(
                        out=S[:],
                        in0=dstl_t[:, pc + j:pc + j + 1].to_broadcast([P, P]),
                        in1=iotab_t[:], op=mybir.AluOpType.is_equal)
                    nc.tensor.matmul(
                        out=pacc[:], lhsT=S[:],
                        rhs=he[:, (1 + j) * C1:(2 + j) * C1],
                        start=(j == 0), stop=False)
                pc += nch
                # pacc += self rows; pacc += b1/dinv (so relu(dinv*pacc) is exact)
                nc.tensor.matmul(out=pacc[:], lhsT=identb[:], rhs=he[:, 0:C1],
                                 start=False, stop=False)
                nc.tensor.matmul(out=pacc[:], lhsT=rdk_t[:, k * P:(k + 1) * P],
                                 rhs=b1t[0:1, :], start=False, stop=True)
                t4 = tpool.tile([P, C1], f32, tag="t4", name=f"t4_{k}")
                nc.scalar.activation(out=t4[:], in_=pacc[:],
                                     func=mybir.ActivationFunctionType.Relu,
                                     scale=dinvK[:, k:k + 1])
                # h2 = (t4 @ W2) * dinv
                ptr2 = ptrp.tile([P, P], f32, space="PSUM", tag="ptr",
                                 name=f"ptr2_{k}")
                nc.tensor.transpose(out=ptr2[:], in_=t4[:], identity=ident[:])
                hT = tpool.tile([P, P], f32, tag="hT", name=f"hT_{k}")
                nc.vector.tensor_copy(out=hT[:], in_=ptr2[:])
                ph2 = pmmp.tile([P, C2], f32, space="PSUM", tag="pmm",
                                name=f"ph2_{k}")
                nc.tensor.matmul(out=ph2[:], lhsT=hT[:], rhs=W2t[:],
                                 start=True, stop=True)
                nc.scalar.activation(out=h2stash[:, k * C2:(k + 1) * C2],
                                     in_=ph2[:],
                                     func=mybir.ActivationFunctionType.Copy,
                                     scale=dinvK[:, k:k + 1])
                nc.sync.dma_start(out=ag2[k * P:(k + 1) * P, :],
                                  in_=h2stash[:, k * C2:(k + 1) * C2])

            nc.gpsimd.collective_compute(
                "AllGather", mybir.AluOpType.bypass,
                replica_groups=[list(range(CORES))],
                ins=[ag2.opt()], outs=[table2.opt()])

            # ---------- conv2 ----------
            o2 = 0
            pc = 0
            for k in range(TPC):
                nch = nch_of_k[k]
                nidx = nch * P
                he = he2pool.tile([P, NCH2 * C2], f32, tag="he2",
                                  name=f"he2_{k}")
                nc.gpsimd.dma_gather(
                    out_ap=he[:, 0:nch * C2].rearrange(
                        "p (n c) -> p n c", c=C2),
                    in_ap=table2[T2_MID:, :],
                    idxs_ap=gidx2_t[:, o2 // 16:(o2 + nidx) // 16],
                    num_idxs=nidx, num_idxs_reg=nidx, elem_size=C2,
                    single_packet=False, queue_num=next_q())
                o2 += nidx
                pacc = paccp.tile([P, C2], f32, space="PSUM", tag="pacc",
                                  name=f"pacc2_{k}")
                for j in range(nch):
                    S = spool.tile([P, P], f32, tag="S2", name=f"S2_{k}_{j}")
                    nc.vector.tensor_tensor(
                        out=S[:],
                        in0=dstlf_t[:, pc + j:pc + j + 1].to_broadcast([P, P]),
                        in1=iota_t[:], op=mybir.AluOpType.is_equal)
                    nc.tensor.matmul(
                        out=pacc[:], lhsT=S[:],
                        rhs=he[:, j * C2:(j + 1) * C2],
                        start=(j == 0), stop=False)
                pc += nch
                nc.tensor.matmul(out=pacc[:], lhsT=ident[:],
                                 rhs=h2stash[:, k * C2:(k + 1) * C2],
                                 start=False, stop=False)
                nc.tensor.matmul(out=pacc[:], lhsT=rdk_t[:, k * P:(k + 1) * P],
                                 rhs=b2t[0:1, :], start=False, stop=True)
                t4 = tpool.tile([P, C2], f32, tag="u4", name=f"u4_{k}")
                nc.scalar.activation(out=t4[:], in_=pacc[:],
                                     func=mybir.ActivationFunctionType.Relu,
                                     scale=dinvK[:, k:k + 1])
                nc.sync.dma_start(out=y[k * P:(k + 1) * P, :], in_=t4[:])

    nc.compile()
    return nc


_cache = {}


def kernel(x, edge_index, emb_a, emb_b, W1, b1, W2, b2):
    in_maps, meta = prep(x, edge_index, emb_a, emb_b, W1, b1, W2, b2)
    key = (meta["nch_of_k"], meta["NPAIRS"])
    if key not in _cache:
        _cache[key] = build(meta)
    nc = _cache[key]
    res = run_bass_kernel_spmd(nc, in_maps, core_ids=list(range(CORES)))
    out = np.zeros((N, C2), dtype=np.float32)
    for c in range(CORES):
        yc = res.results[c]["y"]
        nodes = np.concatenate(
            [t * P + np.arange(P) for t in meta["core_tiles"][c]])
        valid = nodes < N
        out[nodes[valid]] = yc[valid]
    return out


# revision 13
# speedup vs baseline: 1.1208x; 1.0063x over previous
"""Self-contained GCN encoder kernel for 8 TRN2 NeuronCores (Bass/Tile).

kernel(**inputs) takes the FULL unsharded inputs (as from setup_inputs())
and returns the FULL [50000, 64] float32 output.

Strategy: stage 1 (embedding + W1) is REPLICATED on every core via a fused
host-precomputed lookup table emb_ab = emb_a@W1[:64] (+) emb_b@W1[64:128]
(one dma_gather per 8-tile group, accumulated into PSUM with an
identity-matmul, plus the numeric-feature matmul), writing the full
dinv-scaled h1 table to local DRAM -- no first AllGather, so the slow
startup CC barrier overlaps compute.  Conv aggregations shard dst-node
tiles across cores (LPT-balanced, quantile-matched slot order keeps the
SPMD stream identical); per-edge rows are fetched with dma_gather striped
over 4 SWDGE queues (4x descriptor-generation throughput) using a
mid-table base pointer and signed int16 indices (no A/B table split).
Seg-reduction is one-hot (is_equal) S-matrices x gathered rows on the
TensorEngine into PSUM; the symmetric norm is folded into table rows (src)
and the epilogue scale (dst); conv1 self-loop rows ride along as an extra
gather chunk, conv2 self-loop terms are stashed in SBUF from the conv1
epilogue.  One AllGather (h2 table) runs between the convs.
"""
import numpy as np
from concourse import bacc, mybir, tile
from concourse.bass_utils import run_bass_kernel_spmd
from concourse.masks import make_identity

P = 128
CORES = 8
N = 50000
NTILES = 392
NPAD = NTILES * P      # 50176
TPC = NTILES // CORES  # 49
NLOC = TPC * P         # 6272
C1 = 128
C2 = 64
EMB_MID = 25000
T1_MID = NPAD // 2     # 25088
T2_MID = NPAD // 2
PAD_DSTL = 30000.0
GS = 8                 # tiles per stage-1 gather op
NQ = 4                 # SWDGE queues

f32 = mybir.dt.float32
bf16 = mybir.dt.bfloat16
i16 = mybir.dt.int16


def wrap_idx(arr):
    return arr.reshape(-1, 16).T


def rup(x, m):
    return int((x + m - 1) // m * m)


def prep(x, edge_index, emb_a, emb_b, W1, b1, W2, b2):
    import ml_dtypes
    x = np.asarray(x)
    src, dst = np.asarray(edge_index[0]).astype(np.int64), \
        np.asarray(edge_index[1]).astype(np.int64)
    deg = np.bincount(dst, minlength=N).astype(np.float32) + 1.0
    dinv = np.ones(NPAD, dtype=np.float32)
    dinv[:N] = 1.0 / np.sqrt(deg)

    # ---- tile -> core assignment (LPT on edge counts) ----
    t_of_e = dst // P
    tile_cnt = np.bincount(t_of_e, minlength=NTILES)
    order = np.argsort(-tile_cnt, kind="stable")
    core_loads = np.zeros(CORES, dtype=np.int64)
    core_tiles = [[] for _ in range(CORES)]
    for t in order:
        c = int(np.argmin(core_loads))
        core_tiles[c].append(int(t))
        core_loads[c] += tile_cnt[t]
    c_of_t = np.zeros(NTILES, dtype=np.int64)
    k_of_t = np.zeros(NTILES, dtype=np.int64)
    for c in range(CORES):
        for k, t in enumerate(core_tiles[c]):
            c_of_t[t] = c
            k_of_t[t] = k

    node_ids = np.arange(NPAD)
    trow2 = c_of_t[node_ids // P] * NLOC + k_of_t[node_ids // P] * P \
        + node_ids % P

    # ---- sort edges by (core, slot) ----
    key = c_of_t[t_of_e] * TPC + k_of_t[t_of_e]
    sort = np.argsort(key, kind="stable")
    src_s = src[sort]
    trow2_s = trow2[src_s]
    dstl_s = (dst % P).astype(np.float32)[sort]
    bounds = np.searchsorted(key[sort], np.arange(CORES * TPC + 1))

    # ---- op schedule: one op per slot k; nch = max over cores ----
    nch_of_k = []
    for k in range(TPC):
        m = max(int(bounds[c * TPC + k + 1] - bounds[c * TPC + k])
                for c in range(CORES))
        nch_of_k.append(max(1, rup(m, P) // P))
    NPAIRS = sum(nch_of_k)
    NCH1 = max(nch_of_k) + 1      # +1 self chunk
    NCH2 = max(nch_of_k)
    G1COLS = sum((1 + nch) * P for nch in nch_of_k) // 16
    G2COLS = sum(nch * P for nch in nch_of_k) // 16

    # ---- per-core gather idx / dstl arrays ----
    in_maps = []
    iota = np.tile(np.arange(P, dtype=np.float32)[None, :], (P, 1))

    codes_a = np.zeros(NPAD, dtype=np.int64)
    codes_a[:N] = x[:, 0].astype(np.int64)
    codes_b = np.zeros(NPAD, dtype=np.int64)
    codes_b[:N] = x[:, 1].astype(np.int64)
    # stage-1 idx list: 49 ops x GS tiles; idx = cat_a (>=0, no trailing issue)
    eidx = np.tile(wrap_idx(codes_a.astype(np.int16)), (8, 1))

    # small gather table: emb_a@W1lo [1000, 128]; emb_b part via one-hot matmul
    emb_aw = (np.asarray(emb_a, np.float32)
              @ np.asarray(W1, np.float32)[0:64]).astype(ml_dtypes.bfloat16)
    W1Bp = (np.asarray(emb_b, np.float32)
            @ np.asarray(W1, np.float32)[64:128]).astype(ml_dtypes.bfloat16)
    xbT_rep = np.tile(codes_b.astype(np.float32)[None, :],
                      (50, 1)).astype(ml_dtypes.bfloat16)
    iotap = np.arange(P, dtype=np.float32)[:, None].astype(ml_dtypes.bfloat16)

    xT = np.zeros((8, NPAD), dtype=np.float32)
    xT[:, :N] = x[:, 2:10].T
    xT = xT.astype(ml_dtypes.bfloat16)

    dinv_all = dinv.reshape(NTILES, P).T.copy()   # [P, NTILES]

    for c in range(CORES):
        g1 = np.zeros(G1COLS * 16, dtype=np.int64)
        g2 = np.zeros(G2COLS * 16, dtype=np.int64)
        dstlm = np.full((P, NPAIRS), PAD_DSTL, dtype=np.float32)
        dstlm2 = np.full((P, NPAIRS), PAD_DSTL, dtype=np.float32)
        o1 = o2 = 0
        pc = 0
        for k in range(TPC):
            nch = nch_of_k[k]
            t = core_tiles[c][k]
            # conv1 self chunk: own tile rows
            g1[o1:o1 + P] = t * P + np.arange(P) - T1_MID
            lo, hi = bounds[c * TPC + k], bounds[c * TPC + k + 1]
            m = int(hi - lo)
            i1 = np.zeros(nch * P, dtype=np.int64)
            i2 = np.zeros(nch * P, dtype=np.int64)
            dl = np.full(nch * P, PAD_DSTL, dtype=np.float32)
            dl2 = np.full(nch * P, PAD_DSTL, dtype=np.float32)
            # sort each conv's edges by its table row for HBM locality
            so1 = np.argsort(src_s[lo:hi], kind="stable")
            so2 = np.argsort(trow2_s[lo:hi], kind="stable")
            i1[:m] = (src_s[lo:hi] - T1_MID)[so1]
            dl[:m] = dstl_s[lo:hi][so1]
            i2[:m] = (trow2_s[lo:hi] - T2_MID)[so2]
            dl2[:m] = dstl_s[lo:hi][so2]
            # ensure last wrapped element (list[-1]) is >= 0 in both lists
            if i1[-1] < 0:
                ok = np.where(i1 >= 0)[0]
                assert len(ok), "no safe trailing idx in op (conv1)"
                p_ = int(ok[0])
                for arr in (i1, dl):
                    arr[p_], arr[-1] = arr[-1], arr[p_]
            if i2[-1] < 0:
                ok = np.where(i2 >= 0)[0]
                assert len(ok), "no safe trailing idx in op (conv2)"
                p_ = int(ok[0])
                for arr in (i2, dl2):
                    arr[p_], arr[-1] = arr[-1], arr[p_]
            g1[o1 + P:o1 + P + nch * P] = i1
            g2[o2:o2 + nch * P] = i2
            for j in range(nch):
                dstlm[:, pc + j] = dl[j * P:(j + 1) * P]
                dstlm2[:, pc + j] = dl2[j * P:(j + 1) * P]
            o1 += (1 + nch) * P
            o2 += nch * P
            pc += nch
        assert o1 == G1COLS * 16 and o2 == G2COLS * 16 and pc == NPAIRS

        # self-chunk trailing check: self idx can be negative only if the
        # slot's op list ends with it -- never (edge chunks follow; nch>=1)
        gidx1 = np.tile(wrap_idx(g1.astype(np.int16)), (8, 1))
        gidx2 = np.tile(wrap_idx(g2.astype(np.int16)), (8, 1))

        nodes_own = np.concatenate(
            [t * P + np.arange(P) for t in core_tiles[c]])
        dinvk = dinv[nodes_own].reshape(TPC, P).T.copy()

        rdk = (1.0 / dinvk).reshape(1, -1, order="F").astype(np.float32)

        in_maps.append({
            "emb_aw": emb_aw, "xbT_rep": xbT_rep,
            "iotap": iotap,
            "wcomb": np.concatenate([
                W1Bp.astype(np.float32),
                np.zeros((14, C1), np.float32),
                np.asarray(W1, np.float32)[128:136]]).astype(ml_dtypes.bfloat16),
            "xT": xT,
            "eidx": eidx.copy(),
            "gidx1": gidx1,
            "gidx2": gidx2,
            "dstlm": dstlm.astype(ml_dtypes.bfloat16), "dstlf": dstlm2,
            "dinv_all": dinv_all,
            "dinvk": dinvk,
            "W2": np.asarray(W2, dtype=np.float32),
            "b1f": np.tile(np.asarray(b1, np.float32)[None, :], (P, 1)),
            "b2f": np.tile(np.asarray(b2, np.float32)[None, :], (P, 1)),
            "iota": iota, "iotab": iota.astype(ml_dtypes.bfloat16), "rdk": rdk,
        })

    meta = {"nch_of_k": tuple(nch_of_k), "NPAIRS": NPAIRS, "NCH1": NCH1,
            "NCH2": NCH2, "G1COLS": G1COLS, "G2COLS": G2COLS,
            "core_tiles": core_tiles}
    return in_maps, meta


def build(meta):
    nch_of_k = meta["nch_of_k"]
    NPAIRS = meta["NPAIRS"]
    NCH1 = meta["NCH1"]
    NCH2 = meta["NCH2"]
    G1COLS = meta["G1COLS"]
    G2COLS = meta["G2COLS"]
    ECOLS = NTILES * P // 16

    nc = bacc.Bacc("TRN2", target_bir_lowering=False, debug=False,
                   num_devices=CORES, num_swdge_queues=NQ)
    emb_aw = nc.dram_tensor("emb_aw", [1000, C1], bf16, kind="ExternalInput")
    wcomb = nc.dram_tensor("wcomb", [72, C1], bf16, kind="ExternalInput")
    xbT_rep = nc.dram_tensor("xbT_rep", [50, NPAD], bf16, kind="ExternalInput")
    iotap = nc.dram_tensor("iotap", [P, 1], bf16, kind="ExternalInput")
    xT = nc.dram_tensor("xT", [8, NPAD], bf16, kind="ExternalInput")
    eidx = nc.dram_tensor("eidx", [P, ECOLS], i16, kind="ExternalInput")
    gidx1 = nc.dram_tensor("gidx1", [P, G1COLS], i16, kind="ExternalInput")
    gidx2 = nc.dram_tensor("gidx2", [P, G2COLS], i16, kind="ExternalInput")
    dstlm = nc.dram_tensor("dstlm", [P, NPAIRS], bf16, kind="ExternalInput")
    iotab = nc.dram_tensor("iotab", [P, P], bf16, kind="ExternalInput")
    dstlf = nc.dram_tensor("dstlf", [P, NPAIRS], f32, kind="ExternalInput")
    rdk = nc.dram_tensor("rdk", [1, NLOC], f32, kind="ExternalInput")
    dinv_all = nc.dram_tensor("dinv_all", [P, NTILES], f32, kind="ExternalInput")
    dinvk = nc.dram_tensor("dinvk", [P, TPC], f32, kind="ExternalInput")
    W2 = nc.dram_tensor("W2", [C1, C2], f32, kind="ExternalInput")
    b1f = nc.dram_tensor("b1f", [P, C1], f32, kind="ExternalInput")
    b2f = nc.dram_tensor("b2f", [P, C2], f32, kind="ExternalInput")
    iota = nc.dram_tensor("iota", [P, P], f32, kind="ExternalInput")
    y = nc.dram_tensor("y", [NLOC, C2], f32, kind="ExternalOutput")

    with tile.TileContext(nc) as tc:
        with tc.tile_pool(name="const", bufs=1) as cpool, \
             tc.tile_pool(name="meta", bufs=1) as mpool, \
             tc.tile_pool(name="ge", bufs=10) as gepool, \
             tc.tile_pool(name="xt", bufs=2) as xtpool, \
             tc.tile_pool(name="he1", bufs=7) as he1pool, \
             tc.tile_pool(name="he2", bufs=7) as he2pool, \
             tc.tile_pool(name="sel", bufs=4) as spool, \
             tc.tile_pool(name="epi", bufs=3) as tpool, \
             tc.tile_pool(name="stash", bufs=1) as stpool, \
             tc.tile_pool(name="ptr", bufs=1, space="PSUM") as ptrp, \
             tc.tile_pool(name="pmm", bufs=2, space="PSUM") as pmmp, \
             tc.tile_pool(name="pacc", bufs=5, space="PSUM") as paccp, \
             tc.tile_pool(name="dram", bufs=1, space="DRAM") as dram:

            # ---------- constants ----------
            ident = cpool.tile([P, P], f32, tag="ident")
            make_identity(nc, ident[:])
            identb = cpool.tile([P, P], bf16, tag="identb")
            nc.vector.tensor_copy(out=identb[:], in_=ident[:])
            iota_t = cpool.tile([P, P], f32, tag="iota")
            nc.sync.dma_start(out=iota_t[:], in_=iota[:])
            iotab_t = cpool.tile([P, P], bf16, tag="iotab")
            nc.sync.dma_start(out=iotab_t[:], in_=iotab[:])
            iotap_t = cpool.tile([P, 1], bf16, tag="iotap")
            nc.sync.dma_start(out=iotap_t[:], in_=iotap[:])
            rdk_t = cpool.tile([1, NLOC], f32, tag="rdk")
            nc.sync.dma_start(out=rdk_t[:], in_=rdk[:])
            wcomb_t = cpool.tile([72, C1], bf16, tag="wcomb")
            nc.sync.dma_start(out=wcomb_t[:], in_=wcomb[:])
            W2t = cpool.tile([C1, C2], f32, tag="w2")
            nc.sync.dma_start(out=W2t[:], in_=W2[:])
            b1t = cpool.tile([P, C1], f32, tag="b1")
            nc.sync.dma_start(out=b1t[:], in_=b1f[:])
            b2t = cpool.tile([P, C2], f32, tag="b2")
            nc.sync.dma_start(out=b2t[:], in_=b2f[:])
            dinvA = cpool.tile([P, NTILES], f32, tag="dinvA")
            nc.sync.dma_start(out=dinvA[:], in_=dinv_all[:])
            dinvK = cpool.tile([P, TPC], f32, tag="dinvK")
            nc.sync.dma_start(out=dinvK[:], in_=dinvk[:])
            eidx_t = mpool.tile([P, ECOLS], i16, tag="eidx")
            nc.sync.dma_start(out=eidx_t[:], in_=eidx[:])
            h2stash = stpool.tile([P, TPC * C2], f32, tag="h2stash")

            table1 = dram.tile([NPAD, C1], bf16, tag="table1")
            ag2 = dram.tile([NLOC, C2], f32, tag="ag2")
            table2 = dram.tile([NPAD, C2], f32, tag="table2")

            gq = [0]

            def next_q():
                q = gq[0] % NQ
                gq[0] += 1
                return q

            # ---------- stage 1 (replicated): build full h1 table ----------
            for e in range(NTILES // GS):
                nidx = GS * P
                ge = gepool.tile([P, GS * P], bf16, tag="ge", name=f"ge_{e}")
                nc.gpsimd.dma_gather(
                    out_ap=ge[:].rearrange("p (n c) -> p n c", c=C1),
                    in_ap=emb_aw[:],
                    idxs_ap=eidx_t[:, e * nidx // 16:(e + 1) * nidx // 16],
                    num_idxs=nidx, num_idxs_reg=nidx, elem_size=C1,
                    single_packet=False, queue_num=next_q())
                xb_c = xtpool.tile([50, GS * P], bf16, tag="xb", name=f"xb_{e}")
                nc.sync.dma_start(out=xb_c[:],
                                  in_=xbT_rep[:, e * GS * P:(e + 1) * GS * P])
                comb = gepool.tile([72, GS * P], bf16, tag="ob", name=f"ob_{e}")
                nc.sync.dma_start(out=comb[64:72, :],
                                  in_=xT[:, e * GS * P:(e + 1) * GS * P])
                nc.vector.memset(comb[32:64, :], 0.0)
                nc.vector.tensor_tensor(
                    out=comb[0:50, :], in0=xb_c[:],
                    in1=iotap_t[0:50, 0:1].to_broadcast([50, GS * P]),
                    op=mybir.AluOpType.is_equal)
                h1st = xtpool.tile([P, GS * C1], bf16, tag="h1st",
                                   name=f"h1st_{e}")
                for half in range(GS // 4):
                    w = e * (GS // 4) + half
                    php = pmmp if w % 2 == 0 else paccp
                    wide = php.tile([P, 4 * C1], f32, space="PSUM",
                                    tag="pmm" if w % 2 == 0 else "pacc",
                                    name=f"wide_{w}")
                    nc.tensor.matmul(out=wide[:], lhsT=identb[:],
                                     rhs=ge[:, half * 4 * C1:(half + 1) * 4 * C1],
                                     start=True, stop=False)
                    for jj in range(4):
                        j = half * 4 + jj
                        t = e * GS + j
                        nc.tensor.matmul(
                            out=wide[:, jj * C1:(jj + 1) * C1],
                            lhsT=comb[:, j * P:(j + 1) * P],
                            rhs=wcomb_t[:], start=False, stop=True)
                    for jj in range(4):
                        j = half * 4 + jj
                        t = e * GS + j
                        if t % 2 == 0:
                            nc.scalar.activation(
                                out=h1st[:, j * C1:(j + 1) * C1],
                                in_=wide[:, jj * C1:(jj + 1) * C1],
                                func=mybir.ActivationFunctionType.Copy,
                                scale=dinvA[:, t:t + 1])
                        else:
                            nc.vector.tensor_tensor(
                                out=h1st[:, j * C1:(j + 1) * C1],
                                in0=wide[:, jj * C1:(jj + 1) * C1],
                                in1=dinvA[:, t:t + 1].to_broadcast([P, C1]),
                                op=mybir.AluOpType.mult)
                nc.sync.dma_start(
                    out=table1[e * GS * P:(e + 1) * GS * P, :].rearrange(
                        "(n p) c -> p n c", p=P),
                    in_=h1st[:].rearrange("p (n c) -> p n c", c=C1))

            # conv metadata loads (overlap stage-1)
            gidx1_t = mpool.tile([P, G1COLS], i16, tag="gidx1")
            nc.sync.dma_start(out=gidx1_t[:], in_=gidx1[:])
            gidx2_t = mpool.tile([P, G2COLS], i16, tag="gidx2")
            nc.sync.dma_start(out=gidx2_t[:], in_=gidx2[:])
            dstl_t = mpool.tile([P, NPAIRS], bf16, tag="dstl")
            nc.sync.dma_start(out=dstl_t[:], in_=dstlm[:])
            dstlf_t = mpool.tile([P, NPAIRS], f32, tag="dstlf")
            nc.sync.dma_start(out=dstlf_t[:], in_=dstlf[:])

            tc.strict_bb_all_engine_barrier()

            # ---------- conv1 ----------
            o1 = 0
            pc = 0
            for k in range(TPC):
                nch = nch_of_k[k]
                nidx = (1 + nch) * P
                he = he1pool.tile([P, NCH1 * C1], bf16, tag="he1",
                                  name=f"he1_{k}")
                nc.gpsimd.dma_gather(
                    out_ap=he[:, 0:(1 + nch) * C1].rearrange(
                        "p (n c) -> p n c", c=C1),
                    in_ap=table1[T1_MID:, :],
                    idxs_ap=gidx1_t[:, o1 // 16:(o1 + nidx) // 16],
                    num_idxs=nidx, num_idxs_reg=nidx, elem_size=C1,
                    single_packet=False, queue_num=next_q())
                o1 += nidx
                pacc = paccp.tile([P, C1], f32, space="PSUM", tag="pacc",
                                  name=f"pacc1_{k}")
                for j in range(nch):
                    S = spool.tile([P, P], bf16, tag="S1", name=f"S1_{k}_{j}")
                    nc.vector.tensor_tensor(
                        out=S[:],
                        in0=dstl_t[:, pc + j:pc + j + 1].to_broadcast([P, P]),
                        in1=iotab_t[:], op=mybir.AluOpType.is_equal)
                    nc.tensor.matmul(
                        out=pacc[:], lhsT=S[:],
                        rhs=he[:, (1 + j) * C1:(2 + j) * C1],
                        start=(j == 0), stop=False)
                pc += nch
                # pacc += self rows; pacc += b1/dinv (so relu(dinv*pacc) is exact)
                nc.tensor.matmul(out=pacc[:], lhsT=identb[:], rhs=he[:, 0:C1],
                                 start=False, stop=False)
                nc.tensor.matmul(out=pacc[:], lhsT=rdk_t[:, k * P:(k + 1) * P],
                                 rhs=b1t[0:1, :], start=False, stop=True)
                t4 = tpool.tile([P, C1], f32, tag="t4", name=f"t4_{k}")
                nc.scalar.activation(out=t4[:], in_=pacc[:],
                                     func=mybir.ActivationFunctionType.Relu,
                                     scale=dinvK[:, k:k + 1])
                # h2 = (t4 @ W2) * dinv
                ptr2 = ptrp.tile([P, P], f32, space="PSUM", tag="ptr",
                                 name=f"ptr2_{k}")
                nc.tensor.transpose(out=ptr2[:], in_=t4[:], identity=ident[:])
                hT = tpool.tile([P, P], f32, tag="hT", name=f"hT_{k}")
                nc.vector.tensor_copy(out=hT[:], in_=ptr2[:])
                ph2 = pmmp.tile([P, C2], f32, space="PSUM", tag="pmm",
                                name=f"ph2_{k}")
                nc.tensor.matmul(out=ph2[:], lhsT=hT[:], rhs=W2t[:],
                                 start=True, stop=True)
                nc.scalar.activation(out=h2stash[:, k * C2:(k + 1) * C2],
                                     in_=ph2[:],
                                     func=mybir.ActivationFunctionType.Copy,
                                     scale=dinvK[:, k:k + 1])
                nc.sync.dma_start(out=ag2[k * P:(k + 1) * P, :],
                                  in_=h2stash[:, k * C2:(k + 1) * C2])

            nc.gpsimd.collective_compute(
                "AllGather", mybir.AluOpType.bypass,
                replica_groups=[list(range(CORES))],
                ins=[ag2.opt()], outs=[table2.opt()])

            # ---------- conv2 ----------
            o2 = 0
            pc = 0
            for k in range(TPC):
                nch = nch_of_k[k]
                nidx = nch * P
                he = he2pool.tile([P, NCH2 * C2], f32, tag="he2",
                                  name=f"he2_{k}")
                nc.gpsimd.dma_gather(
                    out_ap=he[:, 0:nch * C2].rearrange(
                        "p (n c) -> p n c", c=C2),
                    in_ap=table2[T2_MID:, :],
                    idxs_ap=gidx2_t[:, o2 // 16:(o2 + nidx) // 16],
                    num_idxs=nidx, num_idxs_reg=nidx, elem_size=C2,
                    single_packet=False, queue_num=next_q())
                o2 += nidx
                pacc = paccp.tile([P, C2], f32, space="PSUM", tag="pacc",
                                  name=f"pacc2_{k}")
                for j in range(nch):
                    S = spool.tile([P, P], f32, tag="S2", name=f"S2_{k}_{j}")
                    nc.vector.tensor_tensor(
                        out=S[:],
                        in0=dstlf_t[:, pc + j:pc + j + 1].to_broadcast([P, P]),
                        in1=iota_t[:], op=mybir.AluOpType.is_equal)
                    nc.tensor.matmul(
                        out=pacc[:], lhsT=S[:],
                        rhs=he[:, j * C2:(j + 1) * C2],
                        start=(j == 0), stop=False)
                pc += nch
                nc.tensor.matmul(out=pacc[:], lhsT=ident[:],
                                 rhs=h2stash[:, k * C2:(k + 1) * C2],
                                 start=False, stop=False)
                nc.tensor.matmul(out=pacc[:], lhsT=rdk_t[:, k * P:(k + 1) * P],
                                 rhs=b2t[0:1, :], start=False, stop=True)
                t4 = tpool.tile([P, C2], f32, tag="u4", name=f"u4_{k}")
                nc.scalar.activation(out=t4[:], in_=pacc[:],
                                     func=mybir.ActivationFunctionType.Relu,
                                     scale=dinvK[:, k:k + 1])
                nc.sync.dma_start(out=y[k * P:(k + 1) * P, :], in_=t4[:])

    nc.compile()
    return nc


_cache = {}


def kernel(x, edge_index, emb_a, emb_b, W1, b1, W2, b2):
    in_maps, meta = prep(x, edge_index, emb_a, emb_b, W1, b1, W2, b2)
    key = (meta["nch_of_k"], meta["NPAIRS"])
    if key not in _cache:
        _cache[key] = build(meta)
    nc = _cache[key]
    res = run_bass_kernel_spmd(nc, in_maps, core_ids=list(range(CORES)))
    out = np.zeros((N, C2), dtype=np.float32)
    for c in range(CORES):
        yc = res.results[c]["y"]
        nodes = np.concatenate(
            [t * P + np.arange(P) for t in meta["core_tiles"][c]])
        valid = nodes < N
        out[nodes[valid]] = yc[valid]
    return out
